# revision 1
# baseline (speedup 1.0000x reference)
"""Trainium2 Bass kernel for a dense transformer block (B=2, T=2048, D=1024, H=16).

Redesign vs baseline:
  - Communication: AllToAll of QKV (head-sharded attention) instead of
    AllGather of activations: 3 collectives (qk, v, y) moving fp8, ~97KB/核
    wire bytes instead of 8MB AllGathers.
  - fp8 (e4m3) DoubleRow matmuls for QKV/proj/fc1/fc2 (256-deep contraction
    per instruction), fp8 plain matmuls for attention scores/AV.
  - x arrives host-pre-transposed (feature-major); LayerNorms computed in
    feature-major layout with bf16 ones-matmul stats on the PE.
  - Softmax exp merged over k-tile pairs ([128,1024] 2-bank PSUM reads),
    denominator via a DoubleRow ones-matmul into row 64 of the y PSUM.

Sharding: core i owns tokens [512i, 512(i+1)) and heads {2i, 2i+1}.
"""

import numpy as np
import ml_dtypes

import concourse.bass as bass
import concourse.mybir as mybir
import concourse.tile as tile
from concourse import bacc
from concourse import bass_utils

F32 = mybir.dt.float32
BF16 = mybir.dt.bfloat16
FP8 = mybir.dt.float8e4
NP_FP8 = ml_dtypes.float8_e4m3fn
NP_BF16 = ml_dtypes.bfloat16

AF = mybir.ActivationFunctionType
OP = mybir.AluOpType
DR = mybir.MatmulPerfMode.DoubleRow

P = 128            # partitions
TB = 512           # tokens per core
D = 1024           # model dim
CT = D // P        # 8 feature tiles
NC = 8             # cores
TOKS = 2 * 2048
FF = 4096
DH = 64
EXP_BIAS = -2.5    # exp(s + EXP_BIAS); cancels in softmax normalization
RG = [list(range(NC))]

_cache: dict = {}


def _ln_stats(nc, pool_sb, pool_ps, src_big, ones_b, epsP, tag):
    """Feature-major LN stats for a [128, CT*TB] fp32 activation tile.

    Returns (rs_bc, m_bc) PSUM [128, TB] f32 broadcast tiles:
    rs_bc = 1/std per token, m_bc = mu/std per token.
    """
    s1 = pool_ps.tile([1, TB], F32, name=f"s1_{tag}", tag="lnrow")
    s2 = pool_ps.tile([1, TB], F32, name=f"s2_{tag}", tag="lnrow")
    for c in range(CT):
        xb = pool_sb.tile([P, TB], BF16, name=f"xb_{tag}_{c}", tag="lnxb")
        nc.gpsimd.tensor_copy(xb[:], src_big[:, c * TB:(c + 1) * TB])
        nc.tensor.matmul(s1[:], ones_b[:, 0:1], xb[:],
                         start=(c == 0), stop=(c == CT - 1))
        sq = pool_sb.tile([P, TB], BF16, name=f"sq_{tag}_{c}", tag="lnsq")
        nc.vector.tensor_mul(sq[:], xb[:], xb[:])
        nc.tensor.matmul(s2[:], ones_b[:, 0:1], sq[:],
                         start=(c == 0), stop=(c == CT - 1))
    mu = pool_sb.tile([1, TB], F32, name=f"mu_{tag}", tag="lnmu")
    nc.vector.tensor_scalar_mul(mu[:], s1[:], 1.0 / D)
    mu2 = pool_sb.tile([1, TB], F32, name=f"mu2_{tag}", tag="lnmu2")
    nc.vector.tensor_mul(mu2[:], mu[:], mu[:])
    vr = pool_sb.tile([1, TB], F32, name=f"vr_{tag}", tag="lnvr")
    nc.vector.scalar_tensor_tensor(out=vr[:], in0=s2[:], scalar=1.0 / D,
                                   in1=mu2[:], op0=OP.mult, op1=OP.subtract)
    sd = pool_sb.tile([1, TB], F32, name=f"sd_{tag}", tag="lnsd")
    nc.scalar.activation(sd[:], vr[:], AF.Sqrt, bias=epsP[0:1, :])
    rs = pool_sb.tile([1, TB], BF16, name=f"rs_{tag}", tag="lnrs")
    mrow = pool_sb.tile([1, TB], BF16, name=f"mr_{tag}", tag="lnmr")
    with nc.allow_low_precision(reason="bf16 LN stats for PE broadcast"):
        nc.vector.reciprocal(rs[:], sd[:])
        nc.vector.tensor_mul(mrow[:], mu[:], rs[:])
    return rs, mrow


def _build():
    nc = bacc.Bacc("TRN2", target_bir_lowering=False, debug=False,
                   enable_asserts=False, num_devices=NC)

    # ---- kernel I/O (per core) ----
    xT = nc.dram_tensor("xT", [D, TB], F32, kind="ExternalInput").ap()
    wqkv = nc.dram_tensor("wqkv", [P, 24 * 4 * 256], FP8,
                          kind="ExternalInput").ap()
    bqkv = nc.dram_tensor("bqkv", [P, 24], F32, kind="ExternalInput").ap()
    wproj = nc.dram_tensor("wproj", [P, 8 * 4 * 256], FP8,
                           kind="ExternalInput").ap()
    wfc1 = nc.dram_tensor("wfc1", [P, 32 * 8 * P], BF16,
                          kind="ExternalInput").ap()
    bfc1 = nc.dram_tensor("bfc1", [P, 32], F32, kind="ExternalInput").ap()
    wfc2 = nc.dram_tensor("wfc2", [P, 32 * 8 * P], BF16,
                          kind="ExternalInput").ap()
    idb = nc.dram_tensor("idb", [P, P], BF16, kind="ExternalInput").ap()
    cmask = nc.dram_tensor("cmask", [P, P], F32, kind="ExternalInput").ap()
    out_t = nc.dram_tensor("out_t", [D, TB], F32, kind="ExternalOutput").ap()

    with tile.TileContext(nc) as tc:
        with (
            tc.tile_pool(name="pers", bufs=1) as pers,
            tc.tile_pool(name="dram", bufs=1, space="DRAM") as dram,
        ):
            # DRAM collective bounce buffers
            qkv_in = [dram.tile([NC, 3 * P, TB // 2], FP8, name=f"qkv_in{hf}",
                                tag=f"qkv_in{hf}") for hf in range(2)]
            qkv_out = [dram.tile([NC, 3 * P, TB // 2], FP8,
                                 name=f"qkv_out{hf}", tag=f"qkv_out{hf}")
                       for hf in range(2)]
            y_in = [dram.tile([NC, DH, TB], FP8, name=f"y_in{h}",
                               tag=f"y_in{h}") for h in range(2)]
            y_out = [dram.tile([NC, DH, TB], FP8, name=f"y_out{h}",
                               tag=f"y_out{h}") for h in range(2)]

            # constants
            ones_b = pers.tile([P, 1], BF16, name="ones_b", tag="ones_b")
            nc.gpsimd.memset(ones_b[:], 1.0)
            ones_rb = pers.tile([1, P], BF16, name="ones_rb", tag="ones_rb")
            nc.gpsimd.memset(ones_rb[:], 1.0)
            ones8 = pers.tile([P, 64], FP8, name="ones8", tag="ones8")
            nc.gpsimd.memset(ones8[:], 1.0)
            epsP = pers.tile([1, 1], F32, name="epsP", tag="epsP")
            nc.gpsimd.memset(epsP[:], 1e-5)
            expb = pers.tile([P, 1], F32, name="expb", tag="expb")
            nc.gpsimd.memset(expb[:], EXP_BIAS)
            idb_sb = pers.tile([P, P], BF16, name="idb_sb", tag="idb_sb")
            cm_sb = pers.tile([P, P], F32, name="cm_sb", tag="cm_sb")
            bq_sb = pers.tile([P, 24], F32, name="bq_sb", tag="bq_sb")
            b1_sb = pers.tile([P, 32], F32, name="b1_sb", tag="b1_sb")

            # persistent activations / weights
            xa = pers.tile([P, CT * TB], F32, name="xa", tag="xa")
            qT = pers.tile([P, NC * TB], FP8, name="qT", tag="qT")
            kT = pers.tile([P, NC * TB], FP8, name="kT", tag="kT")
            vT = pers.tile([P, NC * TB], FP8, name="vT", tag="vT")
            yn_st = pers.tile([P, NC * TB], FP8, name="yn_st", tag="yn_st")
            yall = pers.tile([P, NC * TB], FP8, name="yall", tag="yall")
            x2 = pers.tile([P, CT * TB], F32, name="x2", tag="x2")
            x2m = pers.tile([P, CT * TB], BF16, name="x2m", tag="x2m")
            g1 = pers.tile([P, 32 * TB], BF16, name="g1", tag="g1")
            def dr_w(wt, mt, g):
                """Weight AP for DoubleRow: m-tile mt, 256-group g."""
                off = (mt * 4 + g) * 256
                return wt[:, off:off + 256].rearrange("p (i m) -> p i m", i=2)


            def dr_x(xt, g, width=TB):
                """Activation pair AP: blocks (2g, 2g+1) of a [P, n*TB] tile."""
                off = 2 * g * TB
                return xt[:, off:off + 2 * TB].rearrange(
                    "p (i n) -> p i n", i=2)[:, :, 0:width]

            # ============ Phase A: load x, LN1, QKV, stage + a2a ============
            # pipelined over token halves: stats/apply/QKV/staging for cols
            # [0:256) start while the second half of x is still loading
            HT = TB // 2
            with (
                tc.tile_pool(name="work_a", bufs=3) as work_a,
                tc.tile_pool(name="xn_a", bufs=1) as xn_a,
                tc.tile_pool(name="ps_a", bufs=3, space="PSUM") as ps_a,
                tc.tile_pool(name="ps_ln", bufs=2, space="PSUM") as ps_ln,
                tc.tile_pool(name="ps_bc", bufs=2, space="PSUM") as ps_bc,
                tc.tile_pool(name="ps_tr", bufs=1, space="PSUM") as ps_tr,
            ):
                wq_sb = xn_a.tile([P, 24 * 4 * 256], FP8, name="wq_sb",
                                  tag="wq_sb")
                for half in range(2):
                    hsl = slice(half * HT, (half + 1) * HT)
                    nc.sync.dma_start(
                        xa[:].rearrange("p (c n) -> p c n", c=CT)[:, :, hsl],
                        xT[:].rearrange("(c p) n -> p c n", c=CT)[:, :, hsl])
                nc.sync.dma_start(wq_sb[:], wqkv[:])
                nc.sync.dma_start(bq_sb[:], bqkv[:])
                nc.sync.dma_start(idb_sb[:], idb[:])
                nc.sync.dma_start(cm_sb[:], cmask[:])
                nc.sync.dma_start(b1_sb[:], bfc1[:])
                xn = xn_a.tile([P, CT * TB], FP8, name="xn", tag="xn")
                qkv_st = xn_a.tile([P, NC * 3 * TB], FP8, name="qkv_st",
                                   tag="qkv_st")

                def dr_xh(g, half):
                    off = 2 * g * TB
                    return xn[:, off:off + 2 * TB].rearrange(
                        "p (i n) -> p i n", i=2)[:, :, half * HT:(half + 1) * HT]

                for half in range(2):
                    hsl = slice(half * HT, (half + 1) * HT)
                    # --- LN1 stats over the feature (partition) axis ---
                    s1 = ps_ln.tile([1, HT], F32, name=f"s1a_{half}",
                                    tag="lnrow")
                    s2 = ps_ln.tile([1, HT], F32, name=f"s2a_{half}",
                                    tag="lnrow")
                    for c in range(CT):
                        xb = work_a.tile([P, HT], BF16, name=f"xba{half}_{c}",
                                         tag="lnxb")
                        nc.gpsimd.tensor_copy(
                            xb[:], xa[:, c * TB + half * HT:
                                      c * TB + (half + 1) * HT])
                        nc.tensor.matmul(s1[:], ones_b[:, 0:1], xb[:],
                                         start=(c == 0), stop=(c == CT - 1))
                        sq = work_a.tile([P, HT], BF16, name=f"sqa{half}_{c}",
                                         tag="lnsq")
                        nc.vector.tensor_mul(sq[:], xb[:], xb[:])
                        nc.tensor.matmul(s2[:], ones_b[:, 0:1], sq[:],
                                         start=(c == 0), stop=(c == CT - 1))
                    mu = work_a.tile([1, HT], F32, name=f"mua{half}", tag="lnm")
                    nc.vector.tensor_scalar_mul(mu[:], s1[:], 1.0 / D)
                    mu2 = work_a.tile([1, HT], F32, name=f"m2a{half}",
                                      tag="lnm2")
                    nc.vector.tensor_mul(mu2[:], mu[:], mu[:])
                    vr = work_a.tile([1, HT], F32, name=f"vra{half}", tag="lnv")
                    nc.vector.scalar_tensor_tensor(
                        out=vr[:], in0=s2[:], scalar=1.0 / D, in1=mu2[:],
                        op0=OP.mult, op1=OP.subtract)
                    sd = work_a.tile([1, HT], F32, name=f"sda{half}",
                                     tag="lnsd")
                    nc.scalar.activation(sd[:], vr[:], AF.Sqrt,
                                         bias=epsP[0:1, :])
                    rs = work_a.tile([1, HT], BF16, name=f"rsa{half}",
                                     tag="lnrs")
                    mrow = work_a.tile([1, HT], BF16, name=f"mra{half}",
                                       tag="lnmr")
                    with nc.allow_low_precision(reason="bf16 LN stats"):
                        nc.vector.reciprocal(rs[:], sd[:])
                        nc.vector.tensor_mul(mrow[:], mu[:], rs[:])
                    rs_bc = ps_bc.tile([P, HT], F32, name=f"rsb{half}",
                                       tag="bc")
                    nc.tensor.matmul(rs_bc[:], ones_rb[:], rs[:],
                                     start=True, stop=True)
                    m_bc = ps_bc.tile([P, HT], F32, name=f"mbb{half}",
                                      tag="bc")
                    nc.tensor.matmul(m_bc[:], ones_rb[:], mrow[:],
                                     start=True, stop=True)
                    for c in range(CT):
                        csl = slice(c * TB + half * HT,
                                    c * TB + (half + 1) * HT)
                        t = work_a.tile([P, HT], F32, name=f"ta{half}_{c}",
                                        tag="lnt")
                        nc.vector.tensor_mul(t[:], xa[:, csl], rs_bc[:])
                        nc.vector.tensor_sub(xn[:, csl], t[:], m_bc[:])

                    # --- QKV for this token half ---
                    for tqk in range(3):  # 0=q, 1=k, 2=v
                        for j in range(NC):
                            mt = j * 3 + tqk
                            ps = ps_a.tile([P, HT], F32,
                                           name=f"qkv{half}_{tqk}_{j}",
                                           tag="qkvps")
                            for g in range(4):
                                nc.tensor.matmul(
                                    ps[:], dr_w(wq_sb, mt, g), dr_xh(g, half),
                                    start=(g == 0), stop=(g == 3),
                                    perf_mode=DR)
                            if tqk < 2:
                                dst = qkv_st[:, mt * TB + half * HT:
                                             mt * TB + (half + 1) * HT]
                                if j % 2 == 0:
                                    nc.vector.tensor_scalar_add(
                                        dst, ps[:], bq_sb[:, mt:mt + 1])
                                else:
                                    nc.scalar.activation(
                                        dst, ps[:], AF.Identity,
                                        bias=bq_sb[:, mt:mt + 1])
                            else:
                                vtmp = work_a.tile([P, HT], BF16,
                                                   name=f"vt{half}_{j}",
                                                   tag="vtmp")
                                if j % 2 == 0:
                                    nc.vector.tensor_scalar_add(
                                        vtmp[:], ps[:], bq_sb[:, mt:mt + 1])
                                else:
                                    nc.scalar.activation(
                                        vtmp[:], ps[:], AF.Identity,
                                        bias=bq_sb[:, mt:mt + 1])
                                for blk in range(2):
                                    gblk = half * 2 + blk
                                    ptv = ps_tr.tile(
                                        [P, P], BF16,
                                        name=f"ptv{half}_{j}_{blk}", tag="ptv")
                                    nc.tensor.transpose(
                                        ptv[:], vtmp[:, blk * P:(blk + 1) * P],
                                        idb_sb[:])
                                    off = mt * TB + gblk * P
                                    nc.scalar.copy(
                                        qkv_st[:, off:off + P], ptv[:])
                    # --- stage + AllToAll this token half ---
                    nc.scalar.dma_start(
                        qkv_in[half][:].rearrange("j (r p) n -> p j r n", r=3),
                        qkv_st[:].rearrange(
                            "p (j r n) -> p j r n", j=NC, r=3)[:, :, :, hsl])
                    nc.gpsimd.collective_compute(
                        "AllToAll", OP.bypass, replica_groups=RG,
                        ins=[qkv_in[half][:]], outs=[qkv_out[half][:]])

                # read back q/k/v (gpsimd queue). vT keeps global token-block
                # cols: block gk covers tokens [gk*128, +128) of source
                # gk//4; halves interleave 256-col chunks.
                for hf in range(2):
                    hs2 = slice(hf * HT, (hf + 1) * HT)
                    nc.gpsimd.dma_start(
                        qT[:].rearrange("p (s n) -> p s n", s=NC)[:, :, hs2],
                        qkv_out[hf][:, 0:P, :].rearrange("s p n -> p s n"))
                    nc.gpsimd.dma_start(
                        kT[:].rearrange("p (s n) -> p s n", s=NC)[:, :, hs2],
                        qkv_out[hf][:, P:2 * P, :].rearrange("s p n -> p s n"))
                    nc.gpsimd.dma_start(
                        vT[:].rearrange("p (s n) -> p s n", s=NC)[:, :, hs2],
                        qkv_out[hf][:, 2 * P:3 * P, :].rearrange(
                            "s p n -> p s n"))

            # big weights load late (SBUF freed by phase A pools)
            with (
                tc.tile_pool(name="wlate", bufs=1) as wlate,
                tc.tile_pool(name="w1_p", bufs=3) as w1_p,
                tc.tile_pool(name="w2_p", bufs=3) as w2_p,
            ):
                wp_sb = wlate.tile([P, 8 * 4 * 256], FP8, name="wp_sb",
                                   tag="wp_sb")
                nc.sync.dma_start(wp_sb[:], wproj[:])
                # prefetch the first fc1/fc2 weight chunks during attention
                w1tiles, w2tiles = {}, {}
                for blk in range(3):
                    w1tiles[blk] = w1_p.tile([P, 4 * CT * P], BF16,
                                             name=f"w1t_{blk}", tag="w1t")
                    nc.sync.dma_start(
                        w1tiles[blk][:], wfc1[:, blk * 4 * CT * P:
                                              (blk + 1) * 4 * CT * P])
                    w2tiles[blk] = w2_p.tile([P, 32 * P], BF16,
                                             name=f"w2t_{blk}", tag="w2t")
                    nc.sync.dma_start(
                        w2tiles[blk][:], wfc2[:, blk * 32 * P:
                                              (blk + 1) * 32 * P])

                # ============ Phase B: attention (2 heads, causal) ============
                with (
                    tc.tile_pool(name="ps_pss", bufs=2, space="PSUM") as ps_pss,
                    tc.tile_pool(name="ps_rb", bufs=1, space="PSUM") as ps_rb,
                    tc.tile_pool(name="ps_dn", bufs=1, space="PSUM") as ps_dn,
                    tc.tile_pool(name="ps_yp", bufs=2, space="PSUM") as ps_yp,
                    tc.tile_pool(name="etp_p", bufs=28) as etp_p,
                    tc.tile_pool(name="work_b", bufs=2) as work_b,
                ):
                    for h in range(2):
                        hs = slice(h * DH, (h + 1) * DH)
                        # wave 1: all scores + exp for this head (PE/ACT flow
                        # without blocking on vT); results parked in etp tiles
                        etps = {}
                        for jg in range(NC):
                            b, pj = jg // 4, jg % 4
                            gk0, nfull = 16 * b, 4 * pj
                            for pr in range(nfull // 2 + 2):
                                diag = pr >= nfull // 2
                                gkp = gk0 + 2 * pr
                                pss = ps_pss.tile([P, 2 * TB], F32,
                                                  name=f"pss_{h}_{jg}_{pr}",
                                                  tag="pss")
                                for t in range(2):
                                    gk = gkp + t
                                    nc.tensor.matmul(
                                        pss[:, t * TB:(t + 1) * TB],
                                        kT[hs, gk * P:(gk + 1) * P],
                                        qT[hs, jg * TB:(jg + 1) * TB],
                                        start=True, stop=True,
                                        tile_position=(h * DH, 0))
                                    if diag:
                                        n0 = (2 * pr + t - nfull) * P
                                        nc.vector.tensor_add(
                                            pss[:, t * TB + n0:t * TB + n0 + P],
                                            pss[:, t * TB + n0:t * TB + n0 + P],
                                            cm_sb[:])
                                etp = etp_p.tile([P, 2 * TB], FP8,
                                                 name=f"etp_{h}_{jg}_{pr}",
                                                 tag="etp")
                                lead = 0
                                if diag:
                                    lead = (2 * pr - nfull) * P
                                nc.scalar.activation(etp[:, lead:], 
                                                     pss[:, lead:], AF.Exp,
                                                     bias=expb[:])
                                if diag:
                                    for t in range(2):
                                        n0 = (2 * pr + t - nfull) * P
                                        if n0 > 0:
                                            nc.gpsimd.memset(
                                                etp[:, t * TB:t * TB + n0], 0.0)
                                etps[(jg, pr)] = etp
                        # wave 2: AV accumulation + denominators + normalize
                        for jg in range(NC):
                            b, pj = jg // 4, jg % 4
                            gk0, nfull = 16 * b, 4 * pj
                            npair = nfull // 2 + 2
                            ps_y = ps_yp.tile([DH, TB], F32,
                                              name=f"psy{h}_{jg}", tag="psy")
                            den = ps_dn.tile([32, TB], F32,
                                             name=f"den{h}_{jg}", tag="den")
                            for pr in range(npair):
                                etp = etps[(jg, pr)]
                                first, last = pr == 0, pr == npair - 1
                                voff = (gk0 + 2 * pr) * P
                                nc.tensor.matmul(
                                    ps_y[0:DH, :],
                                    vT[:, voff:voff + 2 * P].rearrange(
                                        "p (i m) -> p i m", i=2)[:, :, hs],
                                    etp[:].rearrange("p (i n) -> p i n", i=2),
                                    start=first, stop=last, perf_mode=DR)
                                nc.tensor.matmul(
                                    den[:],
                                    ones8[:, 0:64].rearrange(
                                        "p (i m) -> p i m", i=2),
                                    etp[:].rearrange("p (i n) -> p i n", i=2),
                                    start=first, stop=last, perf_mode=DR)
                            # normalize: yn = y / den
                            rf = work_b.tile([1, TB], BF16, name=f"rf_{h}_{jg}",
                                             tag="rf")
                            with nc.allow_low_precision(
                                    reason="bf16 softmax denom for broadcast"):
                                nc.vector.reciprocal(rf[:], den[0:1, :])
                            rb = ps_rb.tile([DH, TB], F32, name=f"rb_{h}_{jg}",
                                            tag="rb")
                            nc.tensor.matmul(rb[:], ones_rb[:, 0:DH], rf[:],
                                             start=True, stop=True)
                            rbs = work_b.tile([DH, TB], F32,
                                              name=f"rbs_{h}_{jg}", tag="rbs")
                            nc.vector.tensor_copy(rbs[:], rb[:])
                            nc.vector.tensor_mul(
                                yn_st[h * DH:(h + 1) * DH,
                                      jg * TB:(jg + 1) * TB],
                                ps_y[0:DH, :], rbs[:])
                        # per-head y staging (SP queue; collective launches
                        # as soon as this lands, overlapping the other head)
                        nc.scalar.dma_start(
                            y_in[h][:].rearrange("j p n -> p j n"),
                            yn_st[h * DH:(h + 1) * DH, :].rearrange(
                                "p (j n) -> p j n", j=NC))
                    for h in range(2):
                        nc.gpsimd.collective_compute(
                            "AllToAll", OP.bypass, replica_groups=RG,
                            ins=[y_in[h][:]], outs=[y_out[h][:]])
                        nc.gpsimd.dma_start(
                            yall[h * DH:(h + 1) * DH, :].rearrange(
                                "p (s n) -> p s n", s=NC),
                            y_out[h][:].rearrange("s p n -> p s n"))

                # ======== Phase C: proj + residual + LN2 (token halves) ========
                with (
                    tc.tile_pool(name="work_c", bufs=3) as work_c,
                    tc.tile_pool(name="ps_c", bufs=3, space="PSUM") as ps_c,
                    tc.tile_pool(name="ps_ln2", bufs=2, space="PSUM") as ps_ln2,
                    tc.tile_pool(name="ps_bc2", bufs=2, space="PSUM") as ps_bc2,
                ):
                    for half in range(2):
                        hsl = slice(half * HT, (half + 1) * HT)
                        for c in range(CT):
                            csl = slice(c * TB + half * HT,
                                        c * TB + (half + 1) * HT)
                            ps = ps_c.tile([P, HT], F32,
                                           name=f"proj{half}_{c}",
                                           tag="projps")
                            for g in range(4):
                                nc.tensor.matmul(
                                    ps[:], dr_w(wp_sb, c, g),
                                    dr_x(yall, g)[:, :, hsl],
                                    start=(g == 0), stop=(g == 3),
                                    perf_mode=DR)
                            nc.vector.tensor_add(x2[:, csl], ps[:],
                                                 xa[:, csl])
                        s1 = ps_ln2.tile([1, HT], F32, name=f"s1c_{half}",
                                         tag="lnrow2")
                        s2 = ps_ln2.tile([1, HT], F32, name=f"s2c_{half}",
                                         tag="lnrow2")
                        for c in range(CT):
                            csl = slice(c * TB + half * HT,
                                        c * TB + (half + 1) * HT)
                            xb = work_c.tile([P, HT], BF16,
                                             name=f"xbc{half}_{c}", tag="lnxb")
                            nc.gpsimd.tensor_copy(xb[:], x2[:, csl])
                            nc.tensor.matmul(s1[:], ones_b[:, 0:1], xb[:],
                                             start=(c == 0),
                                             stop=(c == CT - 1))
                            sq = work_c.tile([P, HT], BF16,
                                             name=f"sqc{half}_{c}", tag="lnsq")
                            nc.vector.tensor_mul(sq[:], xb[:], xb[:])
                            nc.tensor.matmul(s2[:], ones_b[:, 0:1], sq[:],
                                             start=(c == 0),
                                             stop=(c == CT - 1))
                        mu = work_c.tile([1, HT], F32, name=f"muc{half}",
                                         tag="lnm")
                        nc.vector.tensor_scalar_mul(mu[:], s1[:], 1.0 / D)
                        mu2 = work_c.tile([1, HT], F32, name=f"m2c{half}",
                                          tag="lnm2")
                        nc.vector.tensor_mul(mu2[:], mu[:], mu[:])
                        vr = work_c.tile([1, HT], F32, name=f"vrc{half}",
                                         tag="lnv")
                        nc.vector.scalar_tensor_tensor(
                            out=vr[:], in0=s2[:], scalar=1.0 / D, in1=mu2[:],
                            op0=OP.mult, op1=OP.subtract)
                        sd = work_c.tile([1, HT], F32, name=f"sdc{half}",
                                         tag="lnsd")
                        nc.scalar.activation(sd[:], vr[:], AF.Sqrt,
                                             bias=epsP[0:1, :])
                        rs = work_c.tile([1, HT], BF16, name=f"rsc{half}",
                                         tag="lnrs")
                        mrow = work_c.tile([1, HT], BF16, name=f"mrc{half}",
                                           tag="lnmr")
                        with nc.allow_low_precision(reason="bf16 LN stats"):
                            nc.vector.reciprocal(rs[:], sd[:])
                            nc.vector.tensor_mul(mrow[:], mu[:], rs[:])
                        rs_bc = ps_bc2.tile([P, HT], F32, name=f"rsb2{half}",
                                            tag="bc2")
                        nc.tensor.matmul(rs_bc[:], ones_rb[:], rs[:],
                                         start=True, stop=True)
                        m_bc = ps_bc2.tile([P, HT], F32, name=f"mbb2{half}",
                                           tag="bc2")
                        nc.tensor.matmul(m_bc[:], ones_rb[:], mrow[:],
                                         start=True, stop=True)
                        for c in range(CT):
                            csl = slice(c * TB + half * HT,
                                        c * TB + (half + 1) * HT)
                            t = work_c.tile([P, HT], F32, name=f"tc{half}_{c}",
                                            tag="lnt2")
                            nc.vector.tensor_mul(t[:], x2[:, csl], rs_bc[:])
                            nc.vector.tensor_sub(x2m[:, csl], t[:], m_bc[:])

                # ============ Phase D: fc1 (bf16) + gelu ============
                with (
                    tc.tile_pool(name="ps_d", bufs=2, space="PSUM") as ps_d,
                ):
                    for blk in range(8):
                        if blk in w1tiles:
                            w1t = w1tiles[blk]
                        else:
                            w1t = w1_p.tile([P, 4 * CT * P], BF16,
                                            name=f"w1t_{blk}", tag="w1t")
                            nc.sync.dma_start(
                                w1t[:], wfc1[:, blk * 4 * CT * P:
                                             (blk + 1) * 4 * CT * P])
                        for hh in range(4):
                            ht = blk * 4 + hh
                            ps = ps_d.tile([P, TB], F32, name=f"fc1_{ht}",
                                           tag="fc1ps")
                            for c in range(CT):
                                woff = (hh * CT + c) * P
                                nc.tensor.matmul(
                                    ps[:], w1t[:, woff:woff + P],
                                    x2m[:, c * TB:(c + 1) * TB],
                                    start=(c == 0), stop=(c == CT - 1))
                            nc.scalar.activation(g1[:, ht * TB:(ht + 1) * TB],
                                                 ps[:], AF.Gelu,
                                                 bias=b1_sb[:, ht:ht + 1])

                # ======== Phase E: fc2 (bf16, c-outer) + residual -> out ======
                with (
                    tc.tile_pool(name="ps_e", bufs=2, space="PSUM") as ps_e,
                    tc.tile_pool(name="x3_p", bufs=2) as x3_p,
                ):
                    for c in range(CT):
                        if c in w2tiles:
                            w2t = w2tiles[c]
                        else:
                            w2t = w2_p.tile([P, 32 * P], BF16,
                                            name=f"w2t_{c}", tag="w2t")
                            nc.sync.dma_start(
                                w2t[:], wfc2[:, c * 32 * P:(c + 1) * 32 * P])
                        ps = ps_e.tile([P, TB], F32, name=f"fc2_{c}",
                                       tag="fc2ps")
                        for ht in range(32):
                            nc.tensor.matmul(
                                ps[:], w2t[:, ht * P:(ht + 1) * P],
                                g1[:, ht * TB:(ht + 1) * TB],
                                start=(ht == 0), stop=(ht == 31))
                        x3 = x3_p.tile([P, TB], F32, name=f"x3_{c}", tag="x3")
                        nc.vector.tensor_add(x3[:], ps[:],
                                             x2[:, c * TB:(c + 1) * TB])
                        nc.sync.dma_start(
                            out_t[c * P:(c + 1) * P, :], x3[:])

    nc.compile()
    return nc


def _pack_dr(W):
    """[K, M] fp32 -> DoubleRow-packed fp8 [128, (K//256)*(M//128)*256],
    ordered m-tile-major: blocks (mt, g) of 256 cols = [i(2), m(128)]."""
    K, M = W.shape
    G, MT = K // 256, M // P
    Wf8 = W.astype(NP_FP8)
    # [g, i, p, mt, m] -> [p, mt, g, i, m]
    t = Wf8.reshape(G, 2, P, MT, P).transpose(2, 3, 0, 1, 4)
    return np.ascontiguousarray(t.reshape(P, MT * G * 2 * P))


def _pack_plain(W, kt_major=False):
    """[K, M] fp32 -> bf16 [128, KT*MT*128].

    kt_major=False: col = mt*(KT*P) + kt*P + m  (fc1: ht-major, contraction
    tiles inner).  kt_major=True: col = kt*(MT*P) + mt*P + m  (fc2: streamed
    by contraction tile, all out-tiles inner)."""
    K, M = W.shape
    KT, MT = K // P, M // P
    r = W.astype(NP_BF16).reshape(KT, P, MT, P)
    if kt_major:
        t = r.transpose(1, 0, 2, 3)
    else:
        t = r.transpose(1, 2, 0, 3)
    return np.ascontiguousarray(t.reshape(P, MT * KT * P))


def _prep_inputs(x, ln1_w, ln1_b, w_qkv, w_proj, ln2_w, ln2_b, w_fc1, w_fc2):
    xf = np.asarray(x, np.float32).reshape(TOKS, D)
    ln1_w = np.asarray(ln1_w, np.float32)
    ln1_b = np.asarray(ln1_b, np.float32)
    ln2_w = np.asarray(ln2_w, np.float32)
    ln2_b = np.asarray(ln2_b, np.float32)

    # fold ln1 scale + softmax scale into w_qkv; reorder columns dest-major
    wq = np.asarray(w_qkv, np.float32).copy()
    wq[:, 0:D] *= 0.125
    wqs = ln1_w[:, None] * wq
    bq = ln1_b @ wq
    w_dev = np.empty((D, 3 * D), np.float32)
    b_dev = np.empty(3 * D, np.float32)
    for j in range(NC):
        for t in range(3):
            mt = j * 3 + t
            src = slice(t * D + j * P, t * D + (j + 1) * P)
            w_dev[:, mt * P:(mt + 1) * P] = wqs[:, src]
            b_dev[mt * P:(mt + 1) * P] = bq[src]
    wqkv_p = _pack_dr(w_dev)
    bqkv_p = np.ascontiguousarray(b_dev.reshape(24, P).T)

    wproj_p = _pack_dr(np.asarray(w_proj, np.float32))
    w1s = ln2_w[:, None] * np.asarray(w_fc1, np.float32)
    wfc1_p = _pack_plain(w1s)
    bfc1_v = ln2_b @ np.asarray(w_fc1, np.float32)
    bfc1_p = np.ascontiguousarray(bfc1_v.reshape(32, P).T)
    wfc2_p = _pack_plain(np.asarray(w_fc2, np.float32))

    idb = np.eye(P, dtype=np.float32).astype(NP_BF16)
    pp, jj = np.meshgrid(np.arange(P), np.arange(P), indexing="ij")
    cm = np.where(pp <= jj, 0.0, -1e30).astype(np.float32)

    common = {
        "wqkv": wqkv_p, "bqkv": bqkv_p, "wproj": wproj_p,
        "wfc1": wfc1_p, "bfc1": bfc1_p, "wfc2": wfc2_p,
        "idb": idb, "cmask": cm,
    }
    in_maps = []
    for i in range(NC):
        m = dict(common)
        m["xT"] = np.ascontiguousarray(xf[TB * i:TB * (i + 1)].T)
        in_maps.append(m)
    return in_maps


def _get_runner():
    if "runner" in _cache:
        return _cache["runner"]
    import jax
    from jax.sharding import Mesh, PartitionSpec, NamedSharding
    from jax.experimental.shard_map import shard_map
    from concourse import bass2jax

    nc = _cache.get("nc")
    if nc is None:
        nc = _cache["nc"] = _build()
    bass2jax.install_neuronx_cc_hook()
    partition_name = nc.partition_id_tensor.name if nc.partition_id_tensor else None
    in_names, out_names, out_avals, zero_outs = [], [], [], []
    for alloc in nc.m.functions[0].allocations:
        if not isinstance(alloc, mybir.MemoryLocationSet):
            continue
        name = alloc.memorylocations[0].name
        if alloc.kind == "ExternalInput":
            if name != partition_name:
                in_names.append(name)
        elif alloc.kind == "ExternalOutput":
            out_names.append(name)
            shape = tuple(alloc.tensor_shape)
            dtype = mybir.dt.np(alloc.dtype)
            out_avals.append(jax.core.ShapedArray(shape, dtype))
            zero_outs.append(np.zeros(shape, dtype))
    n_params = len(in_names)
    all_in_names = in_names + out_names + ([partition_name] if partition_name else [])

    def _body(*args):
        operands = list(args)
        if partition_name is not None:
            operands.append(bass2jax.partition_id_tensor())
        outs = bass2jax._bass_exec_p.bind(
            *operands, out_avals=tuple(out_avals), in_names=tuple(all_in_names),
            out_names=tuple(out_names), lowering_input_output_aliases=(),
            sim_require_finite=True, sim_require_nnan=True, nc=nc)
        return tuple(outs)

    devices = jax.devices()[:NC]
    mesh = Mesh(np.asarray(devices), ("core",))
    nin = n_params + len(out_names)
    sharded = jax.jit(shard_map(
        _body, mesh=mesh, in_specs=(PartitionSpec("core"),) * nin,
        out_specs=(PartitionSpec("core"),) * len(out_names), check_rep=False))
    sh = NamedSharding(mesh, PartitionSpec("core"))
    dev_zeros = [
        jax.device_put(np.zeros((NC * z.shape[0], *z.shape[1:]), z.dtype), sh)
        for z in zero_outs
    ]
    runner = (sharded, in_names, out_names, out_avals, sh, dev_zeros)
    _cache["runner"] = runner
    return runner


def kernel(**inputs):
    import jax
    sharded, in_names, out_names, out_avals, sh, dev_zeros = _get_runner()
    in_maps = _prep_inputs(**inputs)
    concat_in = [np.concatenate([in_maps[c][nm] for c in range(NC)], axis=0)
                 for nm in in_names]
    dev_in = [jax.device_put(a, sh) for a in concat_in]
    out_arrs = sharded(*dev_in, *dev_zeros)
    got = {nm: np.asarray(out_arrs[i]).reshape(NC, *out_avals[i].shape)
           for i, nm in enumerate(out_names)}
    out = np.empty((TOKS, D), np.float32)
    for i in range(NC):
        out[TB * i:TB * (i + 1)] = got["out_t"][i].T
    return out.reshape(2, 2048, D)


if __name__ == "__main__":
    rng = np.random.default_rng(0)
    ins = {
        "x": rng.standard_normal((2, 2048, D), dtype=np.float32),
        "ln1_w": np.ones(D, np.float32),
        "ln1_b": np.zeros(D, np.float32),
        "w_qkv": (rng.standard_normal((D, 3 * D), dtype=np.float32) / 32.0),
        "w_proj": (rng.standard_normal((D, D), dtype=np.float32) / 32.0),
        "ln2_w": np.ones(D, np.float32),
        "ln2_b": np.zeros(D, np.float32),
        "w_fc1": (rng.standard_normal((D, FF), dtype=np.float32) / 32.0),
        "w_fc2": (rng.standard_normal((FF, D), dtype=np.float32) / 64.0),
    }
    out = kernel(**ins)
    print("kernel out", out.shape, out.dtype, float(np.abs(out).mean()))



# revision 14
# speedup vs baseline: 1.1082x; 1.1082x over previous
"""Trainium2 Bass kernel for a dense transformer block (B=2, T=2048, D=1024, H=16).

Redesign vs previous version: the QKV AllToAll (2 collectives + ~70us bubble)
is replaced by replicated head-local QKV compute:
  - Every core loads the FULL x (pre-quantized fp8, feature-major) and
    computes q/k/v for ALL 4096 tokens but only its own 2 heads, directly
    from un-normalized x (fp8 DoubleRow matmuls).
  - LayerNorm1 is folded in algebraically: LN(x)@W = r*(x@W) - r*mu*colsum(W)
    + b. Each core computes LN stats for its own 512 tokens; the per-token
    stat rows [r; -mu; sd] are AllGathered (24KB, ~16us) and applied as a
    rank-2 PSUM-accumulated correction matmul plus one DVE multiply.
  - v is produced directly in [token, feat] layout (x-stationary DoubleRow
    matmul), removing the PE transposes; its r-scale rides a per-partition
    tensor_scalar multiply.
  - q/k stay in bf16 (no wire cost anymore), improving accuracy.

Sharding: core i owns tokens [512i, 512(i+1)) and heads {2i, 2i+1}.
"""

import numpy as np
import ml_dtypes

import concourse.bass as bass
import concourse.mybir as mybir
import concourse.tile as tile
from concourse import bacc
from concourse import bass_utils

F32 = mybir.dt.float32
BF16 = mybir.dt.bfloat16
FP8 = mybir.dt.float8e4
NP_FP8 = ml_dtypes.float8_e4m3fn
NP_BF16 = ml_dtypes.bfloat16

AF = mybir.ActivationFunctionType
OP = mybir.AluOpType
DR = mybir.MatmulPerfMode.DoubleRow

P = 128            # partitions
TB = 512           # tokens per core
D = 1024           # model dim
CT = D // P        # 8 feature tiles
NC = 8             # cores
TOKS = 2 * 2048
FF = 4096
DH = 64
EXP_BIAS = -2.5    # exp(s + EXP_BIAS); cancels in softmax normalization
RG = [list(range(NC))]

_cache: dict = {}


def _build():
    nc = bacc.Bacc("TRN2", target_bir_lowering=False, debug=False,
                   enable_asserts=False, num_devices=NC)

    # ---- kernel I/O (per core) ----
    xT = nc.dram_tensor("xT", [D, TB], F32, kind="ExternalInput").ap()
    xq8 = nc.dram_tensor("xq8", [P, CT * TOKS], FP8,
                         kind="ExternalInput").ap()
    wqk = nc.dram_tensor("wqk", [P, 2 * 4 * 256], FP8,
                         kind="ExternalInput").ap()
    wv = nc.dram_tensor("wv", [P, 4 * 2 * P], FP8, kind="ExternalInput").ap()
    cbqk = nc.dram_tensor("cbqk", [2, 2 * P], BF16,
                          kind="ExternalInput").ap()
    cbv = nc.dram_tensor("cbv", [2, P], BF16, kind="ExternalInput").ap()
    wproj = nc.dram_tensor("wproj", [P, 8 * 4 * 256], FP8,
                           kind="ExternalInput").ap()
    wfc1 = nc.dram_tensor("wfc1", [P, 32 * 8 * P], BF16,
                          kind="ExternalInput").ap()
    bfc1 = nc.dram_tensor("bfc1", [P, 32], F32, kind="ExternalInput").ap()
    wfc2 = nc.dram_tensor("wfc2", [P, 32 * 8 * P], BF16,
                          kind="ExternalInput").ap()
    cmask = nc.dram_tensor("cmask", [P, P], F32, kind="ExternalInput").ap()
    out_t = nc.dram_tensor("out_t", [D, TB], F32, kind="ExternalOutput").ap()

    with tile.TileContext(nc) as tc:
        with (
            tc.tile_pool(name="pers", bufs=1) as pers,
            tc.tile_pool(name="dram", bufs=1, space="DRAM") as dram,
        ):
            # DRAM collective bounce buffers
            st_in = dram.tile([3, TB], BF16, name="st_in", tag="st_in")
            st_out = dram.tile([NC, 3, TB], BF16, name="st_out", tag="st_out")
            y_in = [dram.tile([NC, DH, TB], FP8, name=f"y_in{h}",
                              tag=f"y_in{h}") for h in range(2)]
            y_out = [dram.tile([NC, DH, TB], FP8, name=f"y_out{h}",
                               tag=f"y_out{h}") for h in range(2)]

            # constants
            ones_b = pers.tile([P, 1], BF16, name="ones_b", tag="ones_b")
            nc.gpsimd.memset(ones_b[:], 1.0)
            ones_rb = pers.tile([1, P], BF16, name="ones_rb", tag="ones_rb")
            nc.gpsimd.memset(ones_rb[:], 1.0)
            ones8 = pers.tile([P, 64], FP8, name="ones8", tag="ones8")
            nc.gpsimd.memset(ones8[:], 1.0)
            epsP = pers.tile([1, 1], F32, name="epsP", tag="epsP")
            nc.gpsimd.memset(epsP[:], 1e-5)
            expb = pers.tile([P, 1], F32, name="expb", tag="expb")
            nc.gpsimd.memset(expb[:], EXP_BIAS)
            cm_sb = pers.tile([P, P], F32, name="cm_sb", tag="cm_sb")
            cbqk_sb = pers.tile([2, 2 * P], BF16, name="cbqk_sb",
                                tag="cbqk_sb")
            cbv_sb = pers.tile([2, P], BF16, name="cbv_sb", tag="cbv_sb")
            b1_sb = pers.tile([P, 32], F32, name="b1_sb", tag="b1_sb")

            # persistent activations
            xa = pers.tile([P, CT * TB], F32, name="xa", tag="xa")
            qT = pers.tile([P, TOKS], BF16, name="qT", tag="qT")
            kT = pers.tile([P, TOKS], BF16, name="kT", tag="kT")
            vT = pers.tile([P, TOKS], FP8, name="vT", tag="vT")
            yn_st = pers.tile([P, NC * TB], FP8, name="yn_st", tag="yn_st")
            yall = pers.tile([P, NC * TB], FP8, name="yall", tag="yall")
            x2 = pers.tile([P, CT * TB], F32, name="x2", tag="x2")
            x2m = pers.tile([P, CT * TB], BF16, name="x2m", tag="x2m")
            g1 = pers.tile([P, 32 * TB], BF16, name="g1", tag="g1")

            def dr_w(wt, mt, g):
                """Weight AP for DoubleRow: m-tile mt, 256-group g."""
                off = (mt * 4 + g) * 256
                return wt[:, off:off + 256].rearrange("p (i m) -> p i m", i=2)

            # ============ Phase A: stats + gather + local QKV ============
            with (
                tc.tile_pool(name="pha", bufs=1) as pha,
                tc.tile_pool(name="work_a", bufs=3) as work_a,
                tc.tile_pool(name="stat_a", bufs=1) as stat_a,
            ):
                xq = pha.tile([P, CT * TOKS], FP8, name="xq", tag="xq")
                wqk_sb = pha.tile([P, 2 * 4 * 256], FP8, name="wqk_sb",
                                  tag="wqk_sb")
                wv_sb = pha.tile([P, 4 * 2 * P], FP8, name="wv_sb",
                                 tag="wv_sb")

                nc.sync.dma_start(
                    xa[:].rearrange("p (c n) -> p c n", c=CT),
                    xT[:].rearrange("(c p) n -> p c n", c=CT))
                for sl in range(2):
                    ssl = slice(sl * (TOKS // 2), (sl + 1) * (TOKS // 2))
                    nc.sync.dma_start(
                        xq[:].rearrange("p (c n) -> p c n", c=CT)[:, :, ssl],
                        xq8[:].rearrange("p (c n) -> p c n", c=CT)[:, :, ssl])
                nc.sync.dma_start(wqk_sb[:], wqk[:])
                nc.sync.dma_start(wv_sb[:], wv[:])
                nc.sync.dma_start(cbqk_sb[:], cbqk[:])
                nc.sync.dma_start(cbv_sb[:], cbv[:])
                nc.sync.dma_start(cm_sb[:], cmask[:])
                nc.sync.dma_start(b1_sb[:], bfc1[:])

                # --- LN1 stats over own 512 tokens ---
                with tc.tile_pool(name="ps_ln", bufs=2,
                                  space="PSUM") as ps_ln:
                    s1 = ps_ln.tile([1, TB], F32, name="s1", tag="lnrow")
                    s2 = ps_ln.tile([1, TB], F32, name="s2", tag="lnrow")
                    for c in range(CT):
                        xb = work_a.tile([P, TB], BF16, name=f"xb{c}",
                                         tag="lnxb")
                        nc.gpsimd.tensor_copy(xb[:],
                                              xa[:, c * TB:(c + 1) * TB])
                        nc.tensor.matmul(s1[:], ones_b[:, 0:1], xb[:],
                                         start=(c == 0), stop=(c == CT - 1))
                        sq = work_a.tile([P, TB], BF16, name=f"sq{c}",
                                         tag="lnsq")
                        nc.vector.tensor_mul(sq[:], xb[:], xb[:])
                        nc.tensor.matmul(s2[:], ones_b[:, 0:1], sq[:],
                                         start=(c == 0), stop=(c == CT - 1))
                    mu = stat_a.tile([1, TB], F32, name="mu", tag="lnmu")
                    nc.vector.tensor_scalar_mul(mu[:], s1[:], 1.0 / D)
                    mu2 = stat_a.tile([1, TB], F32, name="mu2", tag="lnmu2")
                    nc.vector.tensor_mul(mu2[:], mu[:], mu[:])
                    vr = stat_a.tile([1, TB], F32, name="vr", tag="lnvr")
                    nc.vector.scalar_tensor_tensor(
                        out=vr[:], in0=s2[:], scalar=1.0 / D, in1=mu2[:],
                        op0=OP.mult, op1=OP.subtract)
                    sd = stat_a.tile([1, TB], F32, name="sd", tag="lnsd")
                    nc.scalar.activation(sd[:], vr[:], AF.Sqrt,
                                         bias=epsP[0:1, :])
                    # st rows: 0 = -mu, 1 = sd, 2 = r
                    r_nmu = stat_a.tile([1, TB], BF16, name="r_nmu",
                                        tag="r_nmu")
                    r_sd = stat_a.tile([1, TB], BF16, name="r_sd", tag="r_sd")
                    r_r = stat_a.tile([1, TB], BF16, name="r_r", tag="r_r")
                    with nc.allow_low_precision(reason="bf16 LN stats"):
                        nc.vector.tensor_scalar_mul(r_nmu[:], mu[:], -1.0)
                        nc.vector.tensor_copy(r_sd[:], sd[:])
                        nc.vector.reciprocal(r_r[:], sd[:])

                # --- AllGather the stat rows ---
                nc.sync.dma_start(st_in[0:1, :], r_nmu[:])
                nc.sync.dma_start(st_in[1:2, :], r_sd[:])
                nc.sync.dma_start(st_in[2:3, :], r_r[:])
                nc.gpsimd.collective_compute(
                    "AllGather", OP.bypass, replica_groups=RG,
                    ins=[st_in[:]], outs=[st_out[:]])
                rows_ms = pha.tile([2, TOKS], BF16, name="rows_ms",
                                   tag="rows_ms")
                nc.gpsimd.dma_start(
                    rows_ms[:].rearrange("r (s n) -> r s n", s=NC),
                    st_out[:, 0:2, :].rearrange("s r n -> r s n"))
                r_row = pha.tile([1, TOKS], BF16, name="r_row", tag="r_row")
                nc.gpsimd.dma_start(
                    r_row[:].rearrange("r (s n) -> r s n", s=NC),
                    st_out[:, 2:3, :].rearrange("s r n -> r s n"))
                rT_b = pha.tile([P, 32], BF16, name="rT_b", tag="rT_b")
                for bb in range(4):
                    nc.gpsimd.dma_start(
                        rT_b[:].rearrange("p (s b) -> p s b", s=NC)[:, :, bb],
                        st_out[:, 2, :].rearrange("s (b p) -> p s b",
                                                  b=4)[:, :, bb])
                rT = pha.tile([P, 32], F32, name="rT", tag="rT")
                nc.scalar.copy(rT[:], rT_b[:])

                # --- q/k: z matmul + rank-2 correction + r multiply ---
                r_sb = pha.tile([P, TOKS], F32, name="r_sb", tag="r_sb")
                CH = 512
                with (
                    tc.tile_pool(name="ps_r", bufs=1, space="PSUM") as ps_r,
                    tc.tile_pool(name="ps_z", bufs=2, space="PSUM") as ps_z,
                    tc.tile_pool(name="ps_v", bufs=2, space="PSUM") as ps_v,
                ):
                    for ch in range(TOKS // CH):
                        cs = slice(ch * CH, (ch + 1) * CH)
                        rb = ps_r.tile([P, CH], F32, name=f"rb{ch}", tag="rb")
                        nc.tensor.matmul(rb[:], ones_rb[:],
                                         r_row[0:1, cs],
                                         start=True, stop=True)
                        nc.scalar.copy(r_sb[:, cs], rb[:])
                        for mt in range(2):  # 0=q, 1=k
                            z = ps_z.tile([P, CH], F32, name=f"z{mt}_{ch}",
                                          tag="zps")
                            for g in range(4):
                                nc.tensor.matmul(
                                    z[:], dr_w(wqk_sb, mt, g),
                                    xq[:, 2 * g * TOKS:
                                       (2 * g + 2) * TOKS].rearrange(
                                        "p (i n) -> p i n", i=2)[:, :, cs],
                                    start=(g == 0), stop=False, perf_mode=DR)
                            nc.tensor.matmul(
                                z[:], cbqk_sb[:, mt * P:(mt + 1) * P],
                                rows_ms[0:2, cs], start=False, stop=True)
                            dst = qT if mt == 0 else kT
                            nc.vector.tensor_mul(dst[:, cs], z[:],
                                                 r_sb[:, cs])
                        # v for the 8 token blocks of this chunk
                        for bi in range(CH // P):
                            b = ch * (CH // P) + bi
                            ts = slice(b * P, (b + 1) * P)
                            pv = ps_v.tile([P, P], F32, name=f"pv{b}",
                                           tag="pv")
                            for g in range(4):
                                nc.tensor.matmul(
                                    pv[:],
                                    xq[:, 2 * g * TOKS:
                                       (2 * g + 2) * TOKS].rearrange(
                                        "p (i n) -> p i n", i=2)[:, :, ts],
                                    wv_sb[:, g * 2 * P:
                                          (g + 1) * 2 * P].rearrange(
                                        "p (i m) -> p i m", i=2),
                                    start=(g == 0), stop=False, perf_mode=DR)
                            nc.tensor.matmul(
                                pv[:], rows_ms[0:2, ts], cbv_sb[:],
                                start=False, stop=True)
                            nc.vector.tensor_scalar_mul(vT[:, ts], pv[:],
                                                        rT[:, b:b + 1])

            # big weights load late (SBUF freed by phase A pools)
            with (
                tc.tile_pool(name="wlate", bufs=1) as wlate,
                tc.tile_pool(name="w1_p", bufs=3) as w1_p,
                tc.tile_pool(name="w2_p", bufs=3) as w2_p,
            ):
                wp_sb = wlate.tile([P, 8 * 4 * 256], FP8, name="wp_sb",
                                   tag="wp_sb")
                nc.sync.dma_start(wp_sb[:], wproj[:])
                # prefetch the first fc1/fc2 weight chunks during attention
                w1tiles, w2tiles = {}, {}
                for blk in range(3):
                    w1tiles[blk] = w1_p.tile([P, 4 * CT * P], BF16,
                                             name=f"w1t_{blk}", tag="w1t")
                    nc.sync.dma_start(
                        w1tiles[blk][:], wfc1[:, blk * 4 * CT * P:
                                              (blk + 1) * 4 * CT * P])
                    w2tiles[blk] = w2_p.tile([P, 32 * P], BF16,
                                             name=f"w2t_{blk}", tag="w2t")
                    nc.sync.dma_start(
                        w2tiles[blk][:], wfc2[:, blk * 32 * P:
                                              (blk + 1) * 32 * P])

                # ============ Phase B: attention (2 heads, causal) ============
                with (
                    tc.tile_pool(name="ps_pss", bufs=2, space="PSUM") as ps_pss,
                    tc.tile_pool(name="ps_rb", bufs=1, space="PSUM") as ps_rb,
                    tc.tile_pool(name="ps_dn", bufs=1, space="PSUM") as ps_dn,
                    tc.tile_pool(name="ps_yp", bufs=2, space="PSUM") as ps_yp,
                    tc.tile_pool(name="etp_p", bufs=6) as etp_p,
                    tc.tile_pool(name="work_b", bufs=2) as work_b,
                ):
                    for h in range(2):
                        hs = slice(h * DH, (h + 1) * DH)
                        for jg in range(NC):
                            b, pj = jg // 4, jg % 4
                            gk0, nfull = 16 * b, 4 * pj
                            npair = nfull // 2 + 2
                            etps = {}
                            for pr in range(npair):
                                diag = pr >= nfull // 2
                                gkp = gk0 + 2 * pr
                                pss = ps_pss.tile([P, 2 * TB], F32,
                                                  name=f"pss_{h}_{jg}_{pr}",
                                                  tag="pss")
                                for t in range(2):
                                    gk = gkp + t
                                    nc.tensor.matmul(
                                        pss[:, t * TB:(t + 1) * TB],
                                        kT[hs, gk * P:(gk + 1) * P],
                                        qT[hs, jg * TB:(jg + 1) * TB],
                                        start=True, stop=True,
                                        tile_position=(h * DH, 0))
                                    if diag:
                                        n0 = (2 * pr + t - nfull) * P
                                        nc.vector.tensor_add(
                                            pss[:, t * TB + n0:
                                                t * TB + n0 + P],
                                            pss[:, t * TB + n0:
                                                t * TB + n0 + P],
                                            cm_sb[:])
                                etp = etp_p.tile([P, 2 * TB], FP8,
                                                 name=f"etp_{h}_{jg}_{pr}",
                                                 tag="etp")
                                lead = 0
                                if diag:
                                    lead = (2 * pr - nfull) * P
                                nc.scalar.activation(etp[:, lead:],
                                                     pss[:, lead:], AF.Exp,
                                                     bias=expb[:])
                                if diag:
                                    for t in range(2):
                                        n0 = (2 * pr + t - nfull) * P
                                        if n0 > 0:
                                            nc.gpsimd.memset(
                                                etp[:, t * TB:t * TB + n0],
                                                0.0)
                                etps[pr] = etp
                            # AV accumulation + denominator
                            ps_y = ps_yp.tile([DH, TB], F32,
                                              name=f"psy{h}_{jg}", tag="psy")
                            den = ps_dn.tile([32, TB], F32,
                                             name=f"den{h}_{jg}", tag="den")
                            for pr in range(npair):
                                etp = etps[pr]
                                first, last = pr == 0, pr == npair - 1
                                voff = (gk0 + 2 * pr) * P
                                nc.tensor.matmul(
                                    ps_y[0:DH, :],
                                    vT[:, voff:voff + 2 * P].rearrange(
                                        "p (i m) -> p i m", i=2)[:, :, hs],
                                    etp[:].rearrange("p (i n) -> p i n", i=2),
                                    start=first, stop=last, perf_mode=DR)
                                nc.tensor.matmul(
                                    den[:],
                                    ones8[:, 0:64].rearrange(
                                        "p (i m) -> p i m", i=2),
                                    etp[:].rearrange("p (i n) -> p i n", i=2),
                                    start=first, stop=last, perf_mode=DR)
                            # normalize: yn = y / den
                            rf = work_b.tile([1, TB], BF16,
                                             name=f"rf_{h}_{jg}", tag="rf")
                            with nc.allow_low_precision(
                                    reason="bf16 softmax denom"):
                                nc.vector.reciprocal(rf[:], den[0:1, :])
                            rb = ps_rb.tile([DH, TB], F32,
                                            name=f"rb_{h}_{jg}", tag="rbb")
                            nc.tensor.matmul(rb[:], ones_rb[:, 0:DH], rf[:],
                                             start=True, stop=True)
                            rbs = work_b.tile([DH, TB], F32,
                                              name=f"rbs_{h}_{jg}", tag="rbs")
                            nc.vector.tensor_copy(rbs[:], rb[:])
                            nc.vector.tensor_mul(
                                yn_st[h * DH:(h + 1) * DH,
                                      jg * TB:(jg + 1) * TB],
                                ps_y[0:DH, :], rbs[:])
                        # per-head y staging + AllToAll + readback
                        nc.sync.dma_start(
                            y_in[h][:].rearrange("j p n -> p j n"),
                            yn_st[h * DH:(h + 1) * DH, :].rearrange(
                                "p (j n) -> p j n", j=NC))
                        nc.gpsimd.collective_compute(
                            "AllToAll", OP.bypass, replica_groups=RG,
                            ins=[y_in[h][:]], outs=[y_out[h][:]])
                        nc.gpsimd.dma_start(
                            yall[h * DH:(h + 1) * DH, :].rearrange(
                                "p (s n) -> p s n", s=NC),
                            y_out[h][:].rearrange("s p n -> p s n"))

                # ======== Phase C: proj + residual + LN2 (token halves) ======
                HT = TB // 2

                def dr_x(xt, g, width=TB):
                    off = 2 * g * TB
                    return xt[:, off:off + 2 * TB].rearrange(
                        "p (i n) -> p i n", i=2)[:, :, 0:width]

                with (
                    tc.tile_pool(name="work_c", bufs=3) as work_c,
                    tc.tile_pool(name="ps_c", bufs=3, space="PSUM") as ps_c,
                    tc.tile_pool(name="ps_ln2", bufs=2, space="PSUM") as ps_ln2,
                    tc.tile_pool(name="ps_bc2", bufs=2, space="PSUM") as ps_bc2,
                ):
                    for half in range(2):
                        hsl = slice(half * HT, (half + 1) * HT)
                        for c in range(CT):
                            csl = slice(c * TB + half * HT,
                                        c * TB + (half + 1) * HT)
                            ps = ps_c.tile([P, HT], F32,
                                           name=f"proj{half}_{c}",
                                           tag="projps")
                            for g in range(4):
                                nc.tensor.matmul(
                                    ps[:], dr_w(wp_sb, c, g),
                                    dr_x(yall, g)[:, :, hsl],
                                    start=(g == 0), stop=(g == 3),
                                    perf_mode=DR)
                            nc.vector.tensor_add(x2[:, csl], ps[:],
                                                 xa[:, csl])
                        s1 = ps_ln2.tile([1, HT], F32, name=f"s1c_{half}",
                                         tag="lnrow2")
                        s2 = ps_ln2.tile([1, HT], F32, name=f"s2c_{half}",
                                         tag="lnrow2")
                        for c in range(CT):
                            csl = slice(c * TB + half * HT,
                                        c * TB + (half + 1) * HT)
                            xb = work_c.tile([P, HT], BF16,
                                             name=f"xbc{half}_{c}",
                                             tag="lnxb")
                            nc.gpsimd.tensor_copy(xb[:], x2[:, csl])
                            nc.tensor.matmul(s1[:], ones_b[:, 0:1], xb[:],
                                             start=(c == 0),
                                             stop=(c == CT - 1))
                            sq = work_c.tile([P, HT], BF16,
                                             name=f"sqc{half}_{c}",
                                             tag="lnsq")
                            nc.vector.tensor_mul(sq[:], xb[:], xb[:])
                            nc.tensor.matmul(s2[:], ones_b[:, 0:1], sq[:],
                                             start=(c == 0),
                                             stop=(c == CT - 1))
                        mu = work_c.tile([1, HT], F32, name=f"muc{half}",
                                         tag="lnm")
                        nc.vector.tensor_scalar_mul(mu[:], s1[:], 1.0 / D)
                        mu2 = work_c.tile([1, HT], F32, name=f"m2c{half}",
                                          tag="lnm2")
                        nc.vector.tensor_mul(mu2[:], mu[:], mu[:])
                        vr = work_c.tile([1, HT], F32, name=f"vrc{half}",
                                         tag="lnv")
                        nc.vector.scalar_tensor_tensor(
                            out=vr[:], in0=s2[:], scalar=1.0 / D, in1=mu2[:],
                            op0=OP.mult, op1=OP.subtract)
                        sd = work_c.tile([1, HT], F32, name=f"sdc{half}",
                                         tag="lnsd")
                        nc.scalar.activation(sd[:], vr[:], AF.Sqrt,
                                             bias=epsP[0:1, :])
                        rs = work_c.tile([1, HT], BF16, name=f"rsc{half}",
                                         tag="lnrs")
                        mrow = work_c.tile([1, HT], BF16, name=f"mrc{half}",
                                           tag="lnmr")
                        with nc.allow_low_precision(reason="bf16 LN stats"):
                            nc.vector.reciprocal(rs[:], sd[:])
                            nc.vector.tensor_mul(mrow[:], mu[:], rs[:])
                        rs_bc = ps_bc2.tile([P, HT], F32, name=f"rsb2{half}",
                                            tag="bc2")
                        nc.tensor.matmul(rs_bc[:], ones_rb[:], rs[:],
                                         start=True, stop=True)
                        m_bc = ps_bc2.tile([P, HT], F32, name=f"mbb2{half}",
                                           tag="bc2")
                        nc.tensor.matmul(m_bc[:], ones_rb[:], mrow[:],
                                         start=True, stop=True)
                        for c in range(CT):
                            csl = slice(c * TB + half * HT,
                                        c * TB + (half + 1) * HT)
                            t = work_c.tile([P, HT], F32, name=f"tc{half}_{c}",
                                            tag="lnt2")
                            nc.vector.tensor_mul(t[:], x2[:, csl], rs_bc[:])
                            nc.vector.tensor_sub(x2m[:, csl], t[:], m_bc[:])

                # ============ Phase D: fc1 (bf16) + gelu ============
                with (
                    tc.tile_pool(name="ps_d", bufs=2, space="PSUM") as ps_d,
                ):
                    for blk in range(8):
                        if blk in w1tiles:
                            w1t = w1tiles[blk]
                        else:
                            w1t = w1_p.tile([P, 4 * CT * P], BF16,
                                            name=f"w1t_{blk}", tag="w1t")
                            nc.sync.dma_start(
                                w1t[:], wfc1[:, blk * 4 * CT * P:
                                             (blk + 1) * 4 * CT * P])
                        for hh in range(4):
                            ht = blk * 4 + hh
                            ps = ps_d.tile([P, TB], F32, name=f"fc1_{ht}",
                                           tag="fc1ps")
                            for c in range(CT):
                                woff = (hh * CT + c) * P
                                nc.tensor.matmul(
                                    ps[:], w1t[:, woff:woff + P],
                                    x2m[:, c * TB:(c + 1) * TB],
                                    start=(c == 0), stop=(c == CT - 1))
                            nc.scalar.activation(g1[:, ht * TB:(ht + 1) * TB],
                                                 ps[:], AF.Gelu,
                                                 bias=b1_sb[:, ht:ht + 1])

                # ======== Phase E: fc2 (bf16, c-outer) + residual -> out =====
                with (
                    tc.tile_pool(name="ps_e", bufs=2, space="PSUM") as ps_e,
                    tc.tile_pool(name="x3_p", bufs=2) as x3_p,
                ):
                    for c in range(CT):
                        if c in w2tiles:
                            w2t = w2tiles[c]
                        else:
                            w2t = w2_p.tile([P, 32 * P], BF16,
                                            name=f"w2t_{c}", tag="w2t")
                            nc.sync.dma_start(
                                w2t[:], wfc2[:, c * 32 * P:(c + 1) * 32 * P])
                        ps = ps_e.tile([P, TB], F32, name=f"fc2_{c}",
                                       tag="fc2ps")
                        for ht in range(32):
                            nc.tensor.matmul(
                                ps[:], w2t[:, ht * P:(ht + 1) * P],
                                g1[:, ht * TB:(ht + 1) * TB],
                                start=(ht == 0), stop=(ht == 31))
                        x3 = x3_p.tile([P, TB], F32, name=f"x3_{c}", tag="x3")
                        nc.vector.tensor_add(x3[:], ps[:],
                                             x2[:, c * TB:(c + 1) * TB])
                        nc.sync.dma_start(
                            out_t[c * P:(c + 1) * P, :], x3[:])

    nc.compile()
    return nc


def _pack_dr(W):
    """[K, M] fp32 -> DoubleRow-packed fp8 [128, (K//256)*(M//128)*256],
    ordered m-tile-major: blocks (mt, g) of 256 cols = [i(2), m(128)]."""
    K, M = W.shape
    G, MT = K // 256, M // P
    Wf8 = W.astype(NP_FP8)
    # [g, i, p, mt, m] -> [p, mt, g, i, m]
    t = Wf8.reshape(G, 2, P, MT, P).transpose(2, 3, 0, 1, 4)
    return np.ascontiguousarray(t.reshape(P, MT * G * 2 * P))


def _pack_plain(W, kt_major=False):
    """[K, M] fp32 -> bf16 [128, KT*MT*128].

    kt_major=False: col = mt*(KT*P) + kt*P + m  (fc1: ht-major, contraction
    tiles inner).  kt_major=True: col = kt*(MT*P) + mt*P + m  (fc2: streamed
    by contraction tile, all out-tiles inner)."""
    K, M = W.shape
    KT, MT = K // P, M // P
    r = W.astype(NP_BF16).reshape(KT, P, MT, P)
    if kt_major:
        t = r.transpose(1, 0, 2, 3)
    else:
        t = r.transpose(1, 2, 0, 3)
    return np.ascontiguousarray(t.reshape(P, MT * KT * P))


def _prep_inputs(x, ln1_w, ln1_b, w_qkv, w_proj, ln2_w, ln2_b, w_fc1, w_fc2):
    xf = np.asarray(x, np.float32).reshape(TOKS, D)
    ln1_w = np.asarray(ln1_w, np.float32)
    ln1_b = np.asarray(ln1_b, np.float32)
    ln2_w = np.asarray(ln2_w, np.float32)
    ln2_b = np.asarray(ln2_b, np.float32)

    # fold ln1 scale + softmax scale into w_qkv
    wq = np.asarray(w_qkv, np.float32).copy()
    wq[:, 0:D] *= 0.125
    wqs = ln1_w[:, None] * wq
    bq = ln1_b @ wq

    # full x, fp8, c-tile-major feature layout
    xq8_q = xf.astype(NP_FP8)
    xq8 = np.ascontiguousarray(
        xq8_q.reshape(TOKS, CT, P).transpose(2, 1, 0).reshape(P, CT * TOKS))

    wproj_p = _pack_dr(np.asarray(w_proj, np.float32))
    w1s = ln2_w[:, None] * np.asarray(w_fc1, np.float32)
    wfc1_p = _pack_plain(w1s)
    bfc1_v = ln2_b @ np.asarray(w_fc1, np.float32)
    bfc1_p = np.ascontiguousarray(bfc1_v.reshape(32, P).T)
    wfc2_p = _pack_plain(np.asarray(w_fc2, np.float32))

    pp, jj = np.meshgrid(np.arange(P), np.arange(P), indexing="ij")
    cm = np.where(pp <= jj, 0.0, -1e30).astype(np.float32)

    common = {
        "xq8": xq8, "wproj": wproj_p,
        "wfc1": wfc1_p, "bfc1": bfc1_p, "wfc2": wfc2_p,
        "cmask": cm,
    }
    in_maps = []
    for i in range(NC):
        m = dict(common)
        m["xT"] = np.ascontiguousarray(xf[TB * i:TB * (i + 1)].T)
        qsl = slice(i * P, (i + 1) * P)
        ksl = slice(D + i * P, D + (i + 1) * P)
        vsl = slice(2 * D + i * P, 2 * D + (i + 1) * P)
        Wqk = np.concatenate([wqs[:, qsl], wqs[:, ksl]], axis=1)  # [D, 256]
        m["wqk"] = _pack_dr(Wqk)
        cs_qk = Wqk.astype(NP_FP8).astype(np.float32).sum(0)      # [256]
        b_qk = np.concatenate([bq[qsl], bq[ksl]])
        m["cbqk"] = np.ascontiguousarray(
            np.stack([cs_qk, b_qk]).astype(NP_BF16))
        Wv8 = wqs[:, vsl].astype(NP_FP8)                          # [D, 128]
        m["wv"] = np.ascontiguousarray(
            Wv8.reshape(4, 2, P, P).transpose(2, 0, 1, 3).reshape(P, 8 * P))
        m["cbv"] = np.ascontiguousarray(
            np.stack([Wv8.astype(np.float32).sum(0),
                      bq[vsl]]).astype(NP_BF16))
        in_maps.append(m)
    return in_maps


def _get_runner():
    if "runner" in _cache:
        return _cache["runner"]
    import jax
    from jax.sharding import Mesh, PartitionSpec, NamedSharding
    from jax.experimental.shard_map import shard_map
    from concourse import bass2jax

    nc = _cache.get("nc")
    if nc is None:
        nc = _cache["nc"] = _build()
    bass2jax.install_neuronx_cc_hook()
    partition_name = nc.partition_id_tensor.name if nc.partition_id_tensor else None
    in_names, out_names, out_avals, zero_outs = [], [], [], []
    for alloc in nc.m.functions[0].allocations:
        if not isinstance(alloc, mybir.MemoryLocationSet):
            continue
        name = alloc.memorylocations[0].name
        if alloc.kind == "ExternalInput":
            if name != partition_name:
                in_names.append(name)
        elif alloc.kind == "ExternalOutput":
            out_names.append(name)
            shape = tuple(alloc.tensor_shape)
            dtype = mybir.dt.np(alloc.dtype)
            out_avals.append(jax.core.ShapedArray(shape, dtype))
            zero_outs.append(np.zeros(shape, dtype))
    n_params = len(in_names)
    all_in_names = in_names + out_names + ([partition_name] if partition_name else [])

    def _body(*args):
        operands = list(args)
        if partition_name is not None:
            operands.append(bass2jax.partition_id_tensor())
        outs = bass2jax._bass_exec_p.bind(
            *operands, out_avals=tuple(out_avals), in_names=tuple(all_in_names),
            out_names=tuple(out_names), lowering_input_output_aliases=(),
            sim_require_finite=True, sim_require_nnan=True, nc=nc)
        return tuple(outs)

    devices = jax.devices()[:NC]
    mesh = Mesh(np.asarray(devices), ("core",))
    nin = n_params + len(out_names)
    sharded = jax.jit(shard_map(
        _body, mesh=mesh, in_specs=(PartitionSpec("core"),) * nin,
        out_specs=(PartitionSpec("core"),) * len(out_names), check_rep=False))
    sh = NamedSharding(mesh, PartitionSpec("core"))
    dev_zeros = [
        jax.device_put(np.zeros((NC * z.shape[0], *z.shape[1:]), z.dtype), sh)
        for z in zero_outs
    ]
    runner = (sharded, in_names, out_names, out_avals, sh, dev_zeros)
    _cache["runner"] = runner
    return runner


def kernel(**inputs):
    import jax
    sharded, in_names, out_names, out_avals, sh, dev_zeros = _get_runner()
    in_maps = _prep_inputs(**inputs)
    concat_in = [np.concatenate([in_maps[c][nm] for c in range(NC)], axis=0)
                 for nm in in_names]
    dev_in = [jax.device_put(a, sh) for a in concat_in]
    out_arrs = sharded(*dev_in, *dev_zeros)
    got = {nm: np.asarray(out_arrs[i]).reshape(NC, *out_avals[i].shape)
           for i, nm in enumerate(out_names)}
    out = np.empty((TOKS, D), np.float32)
    for i in range(NC):
        out[TB * i:TB * (i + 1)] = got["out_t"][i].T
    return out.reshape(2, 2048, D)


if __name__ == "__main__":
    rng = np.random.default_rng(0)
    ins = {
        "x": rng.standard_normal((2, 2048, D), dtype=np.float32),
        "ln1_w": np.ones(D, np.float32),
        "ln1_b": np.zeros(D, np.float32),
        "w_qkv": (rng.standard_normal((D, 3 * D), dtype=np.float32) / 32.0),
        "w_proj": (rng.standard_normal((D, D), dtype=np.float32) / 32.0),
        "ln2_w": np.ones(D, np.float32),
        "ln2_b": np.zeros(D, np.float32),
        "w_fc1": (rng.standard_normal((D, FF), dtype=np.float32) / 32.0),
        "w_fc2": (rng.standard_normal((FF, D), dtype=np.float32) / 64.0),
    }
    out = kernel(**ins)
    print("kernel out", out.shape, out.dtype, float(np.abs(out).mean()))


# revision 23
# speedup vs baseline: 1.2416x; 1.1203x over previous
"""Trainium2 Bass kernel for a dense transformer block (B=2, T=2048, D=1024, H=16).

Redesign vs previous version: the QKV AllToAll (2 collectives + ~70us bubble)
is replaced by replicated head-local QKV compute:
  - Every core loads the FULL x (pre-quantized fp8, feature-major) and
    computes q/k/v for ALL 4096 tokens but only its own 2 heads, directly
    from un-normalized x (fp8 DoubleRow matmuls).
  - LayerNorm1 is folded in algebraically: LN(x)@W = r*(x@W) - r*mu*colsum(W)
    + b. Each core computes LN stats for its own 512 tokens; the per-token
    stat rows [r; -mu; sd] are AllGathered (24KB, ~16us) and applied as a
    rank-2 PSUM-accumulated correction matmul plus one DVE multiply.
  - v is produced directly in [token, feat] layout (x-stationary DoubleRow
    matmul), removing the PE transposes; its r-scale rides a per-partition
    tensor_scalar multiply.
  - q/k stay in bf16 (no wire cost anymore), improving accuracy.

Sharding: core i owns tokens [512i, 512(i+1)) and heads {2i, 2i+1}.
"""

import numpy as np
import ml_dtypes

import concourse.bass as bass
import concourse.mybir as mybir
import concourse.tile as tile
from concourse import bacc
from concourse import bass_utils

F32 = mybir.dt.float32
BF16 = mybir.dt.bfloat16
FP8 = mybir.dt.float8e4
NP_FP8 = ml_dtypes.float8_e4m3fn
NP_BF16 = ml_dtypes.bfloat16

AF = mybir.ActivationFunctionType
OP = mybir.AluOpType
DR = mybir.MatmulPerfMode.DoubleRow

P = 128            # partitions
TB = 512           # tokens per core
D = 1024           # model dim
CT = D // P        # 8 feature tiles
NC = 8             # cores
TOKS = 2 * 2048
FF = 4096
DH = 64
EXP_BIAS = -2.5    # exp(s + EXP_BIAS); cancels in softmax normalization
RG = [list(range(NC))]

_cache: dict = {}


def _build():
    nc = bacc.Bacc("TRN2", target_bir_lowering=False, debug=False,
                   enable_asserts=False, num_devices=NC)

    # ---- kernel I/O (per core) ----
    xT = nc.dram_tensor("xT", [D, TB], F32, kind="ExternalInput").ap()
    xq8 = nc.dram_tensor("xq8", [P, CT * TOKS], FP8,
                         kind="ExternalInput").ap()
    wqk = nc.dram_tensor("wqk", [P, 2 * 4 * 256], FP8,
                         kind="ExternalInput").ap()
    wv = nc.dram_tensor("wv", [P, 4 * 2 * P], FP8, kind="ExternalInput").ap()
    cbqk = nc.dram_tensor("cbqk", [2, 2 * P], BF16,
                          kind="ExternalInput").ap()
    cbv = nc.dram_tensor("cbv", [2, P], BF16, kind="ExternalInput").ap()
    wproj = nc.dram_tensor("wproj", [P, 8 * 4 * 256], FP8,
                           kind="ExternalInput").ap()
    wfc1 = nc.dram_tensor("wfc1", [P, 32 * 8 * P], BF16,
                          kind="ExternalInput").ap()
    bfc1 = nc.dram_tensor("bfc1", [P, 32], F32, kind="ExternalInput").ap()
    wfc2 = nc.dram_tensor("wfc2", [P, 32 * 8 * P], BF16,
                          kind="ExternalInput").ap()
    idb = nc.dram_tensor("idb", [P, P], BF16, kind="ExternalInput").ap()
    cmask = nc.dram_tensor("cmask", [P, P], BF16, kind="ExternalInput").ap()
    out_t = nc.dram_tensor("out_t", [D, TB], F32, kind="ExternalOutput").ap()

    with tile.TileContext(nc) as tc:
        with (
            tc.tile_pool(name="pers", bufs=1) as pers,
            tc.tile_pool(name="dram", bufs=1, space="DRAM") as dram,
        ):
            # DRAM collective bounce buffers
            st_in = dram.tile([3, TB], BF16, name="st_in", tag="st_in")
            st_out = dram.tile([NC, 3, TB], BF16, name="st_out", tag="st_out")
            y_in = [dram.tile([NC, DH, TB], FP8, name=f"y_in{h}",
                              tag=f"y_in{h}") for h in range(2)]
            y_out = [dram.tile([NC, DH, TB], FP8, name=f"y_out{h}",
                               tag=f"y_out{h}") for h in range(2)]

            # constants
            ones_b = pers.tile([P, 1], BF16, name="ones_b", tag="ones_b")
            nc.gpsimd.memset(ones_b[:], 1.0)
            ones_rb = pers.tile([1, P], BF16, name="ones_rb", tag="ones_rb")
            nc.gpsimd.memset(ones_rb[:], 1.0)
            ones8 = pers.tile([P, 64], FP8, name="ones8", tag="ones8")
            nc.gpsimd.memset(ones8[:], 1.0)
            epsP = pers.tile([1, 1], F32, name="epsP", tag="epsP")
            nc.gpsimd.memset(epsP[:], 1e-5)
            expb = pers.tile([P, 1], F32, name="expb", tag="expb")
            nc.gpsimd.memset(expb[:], EXP_BIAS)
            cm_sb = pers.tile([P, P], BF16, name="cm_sb", tag="cm_sb")
            idb_sb = pers.tile([P, P], BF16, name="idb_sb", tag="idb_sb")
            cbqk_sb = pers.tile([2, 2 * P], BF16, name="cbqk_sb",
                                tag="cbqk_sb")
            cbv_sb = pers.tile([2, P], BF16, name="cbv_sb", tag="cbv_sb")
            b1_sb = pers.tile([P, 32], F32, name="b1_sb", tag="b1_sb")

            # persistent activations
            xa = pers.tile([P, CT * TB], F32, name="xa", tag="xa")
            qT = pers.tile([P, TOKS], BF16, name="qT", tag="qT")
            kT = pers.tile([P, TOKS], BF16, name="kT", tag="kT")
            vT = pers.tile([P, TOKS], FP8, name="vT", tag="vT")
            yn_st = pers.tile([P, NC * TB], FP8, name="yn_st", tag="yn_st")
            yall = pers.tile([P, NC * TB], FP8, name="yall", tag="yall")
            x2 = pers.tile([P, CT * TB], F32, name="x2", tag="x2")
            x2b = pers.tile([P, CT * TB], BF16, name="x2b", tag="x2b")
            x2m = pers.tile([P, CT * TB], BF16, name="x2m", tag="x2m")
            g1 = pers.tile([P, 32 * TB], BF16, name="g1", tag="g1")

            def dr_w(wt, mt, g):
                """Weight AP for DoubleRow: m-tile mt, 256-group g."""
                off = (mt * 4 + g) * 256
                return wt[:, off:off + 256].rearrange("p (i m) -> p i m", i=2)

            # ============ Phase A: stats + gather + local QKV ============
            with (
                tc.tile_pool(name="pha", bufs=1) as pha,
                tc.tile_pool(name="work_a", bufs=3) as work_a,
                tc.tile_pool(name="stat_a", bufs=1) as stat_a,
            ):
                xq = pha.tile([P, CT * TOKS], FP8, name="xq", tag="xq")
                wqk_sb = pha.tile([P, 2 * 4 * 256], FP8, name="wqk_sb",
                                  tag="wqk_sb")
                wv_sb = pha.tile([P, 4 * 2 * P], FP8, name="wv_sb",
                                 tag="wv_sb")

                nc.sync.dma_start(
                    xa[:].rearrange("p (c n) -> p c n", c=CT),
                    xT[:].rearrange("(c p) n -> p c n", c=CT))
                for sl in range(2):
                    ssl = slice(sl * (TOKS // 2), (sl + 1) * (TOKS // 2))
                    nc.sync.dma_start(
                        xq[:].rearrange("p (c n) -> p c n", c=CT)[:, :, ssl],
                        xq8[:].rearrange("p (c n) -> p c n", c=CT)[:, :, ssl])
                nc.sync.dma_start(wqk_sb[:], wqk[:])
                nc.sync.dma_start(wv_sb[:], wv[:])
                nc.sync.dma_start(cbqk_sb[:], cbqk[:])
                nc.sync.dma_start(cbv_sb[:], cbv[:])
                nc.sync.dma_start(cm_sb[:], cmask[:])
                nc.sync.dma_start(idb_sb[:], idb[:])
                nc.sync.dma_start(b1_sb[:], bfc1[:])

                # --- LN1 stats over own 512 tokens ---
                with tc.tile_pool(name="ps_ln", bufs=2,
                                  space="PSUM") as ps_ln:
                    s1 = ps_ln.tile([1, TB], F32, name="s1", tag="lnrow")
                    s2 = ps_ln.tile([1, TB], F32, name="s2", tag="lnrow")
                    for c in range(CT):
                        xb = work_a.tile([P, TB], BF16, name=f"xb{c}",
                                         tag="lnxb")
                        nc.gpsimd.tensor_copy(xb[:],
                                              xa[:, c * TB:(c + 1) * TB])
                        nc.tensor.matmul(s1[:], ones_b[:, 0:1], xb[:],
                                         start=(c == 0), stop=(c == CT - 1))
                        sq = work_a.tile([P, TB], BF16, name=f"sq{c}",
                                         tag="lnsq")
                        nc.vector.tensor_mul(sq[:], xb[:], xb[:])
                        nc.tensor.matmul(s2[:], ones_b[:, 0:1], sq[:],
                                         start=(c == 0), stop=(c == CT - 1))
                    mu = stat_a.tile([1, TB], F32, name="mu", tag="lnmu")
                    nc.vector.tensor_scalar_mul(mu[:], s1[:], 1.0 / D)
                    mu2 = stat_a.tile([1, TB], F32, name="mu2", tag="lnmu2")
                    nc.vector.tensor_mul(mu2[:], mu[:], mu[:])
                    vr = stat_a.tile([1, TB], F32, name="vr", tag="lnvr")
                    nc.vector.scalar_tensor_tensor(
                        out=vr[:], in0=s2[:], scalar=1.0 / D, in1=mu2[:],
                        op0=OP.mult, op1=OP.subtract)
                    sd = stat_a.tile([1, TB], F32, name="sd", tag="lnsd")
                    nc.scalar.activation(sd[:], vr[:], AF.Sqrt,
                                         bias=epsP[0:1, :])
                    # st rows: 0 = -mu, 1 = sd, 2 = r
                    r_nmu = stat_a.tile([1, TB], BF16, name="r_nmu",
                                        tag="r_nmu")
                    r_sd = stat_a.tile([1, TB], BF16, name="r_sd", tag="r_sd")
                    r_r = stat_a.tile([1, TB], BF16, name="r_r", tag="r_r")
                    with nc.allow_low_precision(reason="bf16 LN stats"):
                        nc.vector.tensor_scalar_mul(r_nmu[:], mu[:], -1.0)
                        nc.vector.tensor_copy(r_sd[:], sd[:])
                        nc.vector.reciprocal(r_r[:], sd[:])

                # --- AllGather the stat rows ---
                nc.sync.dma_start(st_in[0:1, :], r_nmu[:])
                nc.sync.dma_start(st_in[1:2, :], r_sd[:])
                nc.sync.dma_start(st_in[2:3, :], r_r[:])
                nc.gpsimd.collective_compute(
                    "AllGather", OP.bypass, replica_groups=RG,
                    ins=[st_in[:]], outs=[st_out[:]])
                rows_ms = pha.tile([2, TOKS], BF16, name="rows_ms",
                                   tag="rows_ms")
                nc.gpsimd.dma_start(
                    rows_ms[:].rearrange("r (s n) -> r s n", s=NC),
                    st_out[:, 0:2, :].rearrange("s r n -> r s n"))
                r_row = pha.tile([1, TOKS], BF16, name="r_row", tag="r_row")
                nc.gpsimd.dma_start(
                    r_row[:].rearrange("r (s n) -> r s n", s=NC),
                    st_out[:, 2:3, :].rearrange("s r n -> r s n"))
                rT_b = pha.tile([P, 32], BF16, name="rT_b", tag="rT_b")
                for bb in range(4):
                    nc.gpsimd.dma_start(
                        rT_b[:].rearrange("p (s b) -> p s b", s=NC)[:, :, bb],
                        st_out[:, 2, :].rearrange("s (b p) -> p s b",
                                                  b=4)[:, :, bb])
                rT = pha.tile([P, 32], F32, name="rT", tag="rT")
                nc.scalar.copy(rT[:], rT_b[:])

                # --- q/k: z matmul + rank-2 correction + r multiply ---
                r_sb = pha.tile([P, TOKS], F32, name="r_sb", tag="r_sb")
                CH = 512
                with (
                    tc.tile_pool(name="ps_r", bufs=1, space="PSUM") as ps_r,
                    tc.tile_pool(name="ps_z", bufs=2, space="PSUM") as ps_z,
                    tc.tile_pool(name="ps_v", bufs=2, space="PSUM") as ps_v,
                ):
                    for ch in range(TOKS // CH):
                        cs = slice(ch * CH, (ch + 1) * CH)
                        rb = ps_r.tile([P, CH], F32, name=f"rb{ch}", tag="rb")
                        nc.tensor.matmul(rb[:], ones_rb[:],
                                         r_row[0:1, cs],
                                         start=True, stop=True)
                        nc.scalar.copy(r_sb[:, cs], rb[:])
                        for mt in range(2):  # 0=q, 1=k
                            z = ps_z.tile([P, CH], F32, name=f"z{mt}_{ch}",
                                          tag="zps")
                            for g in range(4):
                                nc.tensor.matmul(
                                    z[:], dr_w(wqk_sb, mt, g),
                                    xq[:, 2 * g * TOKS:
                                       (2 * g + 2) * TOKS].rearrange(
                                        "p (i n) -> p i n", i=2)[:, :, cs],
                                    start=(g == 0), stop=False, perf_mode=DR)
                            nc.tensor.matmul(
                                z[:], cbqk_sb[:, mt * P:(mt + 1) * P],
                                rows_ms[0:2, cs], start=False, stop=True)
                            dst = qT if mt == 0 else kT
                            nc.vector.tensor_mul(dst[:, cs], z[:],
                                                 r_sb[:, cs])
                        # v for the 8 token blocks of this chunk
                        for bi in range(CH // P):
                            b = ch * (CH // P) + bi
                            ts = slice(b * P, (b + 1) * P)
                            pv = ps_v.tile([P, P], F32, name=f"pv{b}",
                                           tag="pv")
                            for g in range(4):
                                nc.tensor.matmul(
                                    pv[:],
                                    xq[:, 2 * g * TOKS:
                                       (2 * g + 2) * TOKS].rearrange(
                                        "p (i n) -> p i n", i=2)[:, :, ts],
                                    wv_sb[:, g * 2 * P:
                                          (g + 1) * 2 * P].rearrange(
                                        "p (i m) -> p i m", i=2),
                                    start=(g == 0), stop=False, perf_mode=DR)
                            nc.tensor.matmul(
                                pv[:], rows_ms[0:2, ts], cbv_sb[:],
                                start=False, stop=True)
                            nc.vector.tensor_scalar_mul(vT[:, ts], pv[:],
                                                        rT[:, b:b + 1])

            # big weights load late (SBUF freed by phase A pools)
            with (
                tc.tile_pool(name="wlate", bufs=1) as wlate,
                tc.tile_pool(name="w1_p", bufs=3) as w1_p,
                tc.tile_pool(name="w2_p", bufs=3) as w2_p,
            ):
                wp_sb = wlate.tile([P, 8 * 4 * 256], FP8, name="wp_sb",
                                   tag="wp_sb")
                nc.sync.dma_start(wp_sb[:], wproj[:])
                # prefetch the first fc1/fc2 weight chunks during attention
                w1tiles, w2tiles = {}, {}
                for blk in range(3):
                    w1tiles[blk] = w1_p.tile([P, 4 * CT * P], BF16,
                                             name=f"w1t_{blk}", tag="w1t")
                    nc.sync.dma_start(
                        w1tiles[blk][:], wfc1[:, blk * 4 * CT * P:
                                              (blk + 1) * 4 * CT * P])
                    w2tiles[blk] = w2_p.tile([P, 32 * P], BF16,
                                             name=f"w2t_{blk}", tag="w2t")
                    nc.sync.dma_start(
                        w2tiles[blk][:], wfc2[:, blk * 32 * P:
                                              (blk + 1) * 32 * P])

                # ============ Phase B: attention (2 heads, causal) ============
                with (
                    tc.tile_pool(name="ps_pss", bufs=2, space="PSUM") as ps_pss,
                    tc.tile_pool(name="ps_rb", bufs=1, space="PSUM") as ps_rb,
                    tc.tile_pool(name="ps_dn", bufs=1, space="PSUM") as ps_dn,
                    tc.tile_pool(name="ps_yp", bufs=2, space="PSUM") as ps_yp,
                    tc.tile_pool(name="etp_p", bufs=6) as etp_p,
                    tc.tile_pool(name="work_b", bufs=2) as work_b,
                ):
                    for h in range(2):
                        hs = slice(h * DH, (h + 1) * DH)
                        for jg in range(NC):
                            b, pj = jg // 4, jg % 4
                            gk0, nfull = 16 * b, 4 * pj
                            npair = nfull // 2 + 2
                            etps = {}
                            for pr in range(npair):
                                diag = pr >= nfull // 2
                                gkp = gk0 + 2 * pr
                                pss = ps_pss.tile([P, 2 * TB], F32,
                                                  name=f"pss_{h}_{jg}_{pr}",
                                                  tag="pss")
                                for t in range(2):
                                    gk = gkp + t
                                    nc.tensor.matmul(
                                        pss[:, t * TB:(t + 1) * TB],
                                        kT[hs, gk * P:(gk + 1) * P],
                                        qT[hs, jg * TB:(jg + 1) * TB],
                                        start=True, stop=not diag,
                                        tile_position=(h * DH, 0),
                                        skip_group_check=True)
                                    if diag:
                                        n0 = (2 * pr + t - nfull) * P
                                        nc.tensor.matmul(
                                            pss[:, t * TB + n0:
                                                t * TB + n0 + P],
                                            idb_sb[:], cm_sb[:],
                                            start=False, stop=True,
                                            skip_group_check=True)
                                etp = etp_p.tile([P, 2 * TB], FP8,
                                                 name=f"etp_{h}_{jg}_{pr}",
                                                 tag="etp")
                                lead = 0
                                if diag:
                                    lead = (2 * pr - nfull) * P
                                nc.scalar.activation(etp[:, lead:],
                                                     pss[:, lead:], AF.Exp,
                                                     bias=expb[:])
                                if diag:
                                    for t in range(2):
                                        n0 = (2 * pr + t - nfull) * P
                                        if n0 > 0:
                                            nc.vector.memset(
                                                etp[:, t * TB:t * TB + n0],
                                                0.0)
                                etps[pr] = etp
                            # AV accumulation + denominator
                            ps_y = ps_yp.tile([DH, TB], F32,
                                              name=f"psy{h}_{jg}", tag="psy")
                            den = ps_dn.tile([32, TB], F32,
                                             name=f"den{h}_{jg}", tag="den")
                            for pr in range(npair):
                                etp = etps[pr]
                                first, last = pr == 0, pr == npair - 1
                                voff = (gk0 + 2 * pr) * P
                                nc.tensor.matmul(
                                    ps_y[0:DH, :],
                                    vT[:, voff:voff + 2 * P].rearrange(
                                        "p (i m) -> p i m", i=2)[:, :, hs],
                                    etp[:].rearrange("p (i n) -> p i n", i=2),
                                    start=first, stop=last, perf_mode=DR)
                                nc.tensor.matmul(
                                    den[:],
                                    ones8[:, 0:64].rearrange(
                                        "p (i m) -> p i m", i=2),
                                    etp[:].rearrange("p (i n) -> p i n", i=2),
                                    start=first, stop=last, perf_mode=DR)
                            # normalize: yn = y / den
                            rf = work_b.tile([1, TB], BF16,
                                             name=f"rf_{h}_{jg}", tag="rf")
                            with nc.allow_low_precision(
                                    reason="bf16 softmax denom"):
                                nc.vector.reciprocal(rf[:], den[0:1, :])
                            rb = ps_rb.tile([DH, TB], F32,
                                            name=f"rb_{h}_{jg}", tag="rbb")
                            nc.tensor.matmul(rb[:], ones_rb[:, 0:DH], rf[:],
                                             start=True, stop=True)
                            rbs = work_b.tile([DH, TB], F32,
                                              name=f"rbs_{h}_{jg}", tag="rbs")
                            nc.vector.tensor_copy(rbs[:], rb[:])
                            nc.vector.tensor_mul(
                                yn_st[h * DH:(h + 1) * DH,
                                      jg * TB:(jg + 1) * TB],
                                ps_y[0:DH, :], rbs[:])
                        # per-head y staging + AllToAll + readback
                        nc.sync.dma_start(
                            y_in[h][:].rearrange("j p n -> p j n"),
                            yn_st[h * DH:(h + 1) * DH, :].rearrange(
                                "p (j n) -> p j n", j=NC))
                        nc.gpsimd.collective_compute(
                            "AllToAll", OP.bypass, replica_groups=RG,
                            ins=[y_in[h][:]], outs=[y_out[h][:]])
                        nc.gpsimd.dma_start(
                            yall[h * DH:(h + 1) * DH, :].rearrange(
                                "p (s n) -> p s n", s=NC),
                            y_out[h][:].rearrange("s p n -> p s n"))

                # ======== Phase C: proj + residual + LN2 (token halves) ======
                HT = TB // 2

                def dr_x(xt, g, width=TB):
                    off = 2 * g * TB
                    return xt[:, off:off + 2 * TB].rearrange(
                        "p (i n) -> p i n", i=2)[:, :, 0:width]

                with (
                    tc.tile_pool(name="work_c", bufs=3) as work_c,
                    tc.tile_pool(name="ps_c", bufs=3, space="PSUM") as ps_c,
                    tc.tile_pool(name="ps_ln2", bufs=2, space="PSUM") as ps_ln2,
                    tc.tile_pool(name="ps_bc2", bufs=2, space="PSUM") as ps_bc2,
                ):
                    for half in range(2):
                        hsl = slice(half * HT, (half + 1) * HT)
                        for c in range(CT):
                            csl = slice(c * TB + half * HT,
                                        c * TB + (half + 1) * HT)
                            ps = ps_c.tile([P, HT], F32,
                                           name=f"proj{half}_{c}",
                                           tag="projps")
                            for g in range(4):
                                nc.tensor.matmul(
                                    ps[:], dr_w(wp_sb, c, g),
                                    dr_x(yall, g)[:, :, hsl],
                                    start=(g == 0), stop=(g == 3),
                                    perf_mode=DR)
                            nc.vector.tensor_add(x2[:, csl], ps[:],
                                                 xa[:, csl])
                            nc.gpsimd.tensor_copy(x2b[:, csl], x2[:, csl])
                        s1 = ps_ln2.tile([1, HT], F32, name=f"s1c_{half}",
                                         tag="lnrow2")
                        s2 = ps_ln2.tile([1, HT], F32, name=f"s2c_{half}",
                                         tag="lnrow2")
                        for c in range(CT):
                            csl = slice(c * TB + half * HT,
                                        c * TB + (half + 1) * HT)
                            nc.tensor.matmul(s1[:], ones_b[:, 0:1],
                                             x2b[:, csl],
                                             start=(c == 0),
                                             stop=(c == CT - 1))
                            sq = work_c.tile([P, HT], BF16,
                                             name=f"sqc{half}_{c}",
                                             tag="lnsq")
                            nc.vector.tensor_mul(sq[:], x2b[:, csl],
                                                 x2b[:, csl])
                            nc.tensor.matmul(s2[:], ones_b[:, 0:1], sq[:],
                                             start=(c == 0),
                                             stop=(c == CT - 1))
                        mu = work_c.tile([1, HT], F32, name=f"muc{half}",
                                         tag="lnm")
                        nc.vector.tensor_scalar_mul(mu[:], s1[:], 1.0 / D)
                        mu2 = work_c.tile([1, HT], F32, name=f"m2c{half}",
                                          tag="lnm2")
                        nc.vector.tensor_mul(mu2[:], mu[:], mu[:])
                        vr = work_c.tile([1, HT], F32, name=f"vrc{half}",
                                         tag="lnv")
                        nc.vector.scalar_tensor_tensor(
                            out=vr[:], in0=s2[:], scalar=1.0 / D, in1=mu2[:],
                            op0=OP.mult, op1=OP.subtract)
                        sd = work_c.tile([1, HT], F32, name=f"sdc{half}",
                                         tag="lnsd")
                        nc.scalar.activation(sd[:], vr[:], AF.Sqrt,
                                             bias=epsP[0:1, :])
                        rs = work_c.tile([1, HT], BF16, name=f"rsc{half}",
                                         tag="lnrs")
                        mrow = work_c.tile([1, HT], BF16, name=f"mrc{half}",
                                           tag="lnmr")
                        with nc.allow_low_precision(reason="bf16 LN stats"):
                            nc.vector.reciprocal(rs[:], sd[:])
                            nc.vector.tensor_mul(mrow[:], mu[:], rs[:])
                        rs_bc = ps_bc2.tile([P, HT], F32, name=f"rsb2{half}",
                                            tag="bc2")
                        nc.tensor.matmul(rs_bc[:], ones_rb[:], rs[:],
                                         start=True, stop=True)
                        m_bc = ps_bc2.tile([P, HT], F32, name=f"mbb2{half}",
                                           tag="bc2")
                        nc.tensor.matmul(m_bc[:], ones_rb[:], mrow[:],
                                         start=True, stop=True)
                        rs_bb = work_c.tile([P, HT], BF16, name=f"rsbb{half}",
                                            tag="rsbb")
                        nc.scalar.copy(rs_bb[:], rs_bc[:])
                        m_bb = work_c.tile([P, HT], BF16, name=f"mbb{half}",
                                           tag="mbb")
                        nc.scalar.copy(m_bb[:], m_bc[:])
                        with nc.allow_low_precision(reason="bf16 LN2 apply"):
                            for c in range(CT):
                                csl = slice(c * TB + half * HT,
                                            c * TB + (half + 1) * HT)
                                t = work_c.tile([P, HT], BF16,
                                                name=f"tc{half}_{c}",
                                                tag="lnt2")
                                nc.vector.tensor_mul(t[:], x2b[:, csl],
                                                     rs_bb[:])
                                nc.vector.tensor_sub(x2m[:, csl], t[:],
                                                     m_bb[:])

                # ============ Phase D: fc1 (bf16) + gelu ============
                with (
                    tc.tile_pool(name="ps_d", bufs=2, space="PSUM") as ps_d,
                ):
                    for blk in range(8):
                        if blk in w1tiles:
                            w1t = w1tiles[blk]
                        else:
                            w1t = w1_p.tile([P, 4 * CT * P], BF16,
                                            name=f"w1t_{blk}", tag="w1t")
                            nc.sync.dma_start(
                                w1t[:], wfc1[:, blk * 4 * CT * P:
                                             (blk + 1) * 4 * CT * P])
                        for hh in range(4):
                            ht = blk * 4 + hh
                            ps = ps_d.tile([P, TB], F32, name=f"fc1_{ht}",
                                           tag="fc1ps")
                            for c in range(CT):
                                woff = (hh * CT + c) * P
                                nc.tensor.matmul(
                                    ps[:], w1t[:, woff:woff + P],
                                    x2m[:, c * TB:(c + 1) * TB],
                                    start=(c == 0), stop=(c == CT - 1))
                            nc.scalar.activation(g1[:, ht * TB:(ht + 1) * TB],
                                                 ps[:], AF.Gelu,
                                                 bias=b1_sb[:, ht:ht + 1])

                # ======== Phase E: fc2 (bf16, c-outer) + residual -> out =====
                with (
                    tc.tile_pool(name="ps_e", bufs=2, space="PSUM") as ps_e,
                    tc.tile_pool(name="x3_p", bufs=2) as x3_p,
                ):
                    for c in range(CT):
                        if c in w2tiles:
                            w2t = w2tiles[c]
                        else:
                            w2t = w2_p.tile([P, 32 * P], BF16,
                                            name=f"w2t_{c}", tag="w2t")
                            nc.sync.dma_start(
                                w2t[:], wfc2[:, c * 32 * P:(c + 1) * 32 * P])
                        ps = ps_e.tile([P, TB], F32, name=f"fc2_{c}",
                                       tag="fc2ps")
                        for ht in range(32):
                            nc.tensor.matmul(
                                ps[:], w2t[:, ht * P:(ht + 1) * P],
                                g1[:, ht * TB:(ht + 1) * TB],
                                start=(ht == 0), stop=(ht == 31))
                        x3 = x3_p.tile([P, TB], F32, name=f"x3_{c}", tag="x3")
                        nc.vector.tensor_add(x3[:], ps[:],
                                             x2[:, c * TB:(c + 1) * TB])
                        nc.sync.dma_start(
                            out_t[c * P:(c + 1) * P, :], x3[:])

    nc.compile()
    return nc


def _pack_dr(W):
    """[K, M] fp32 -> DoubleRow-packed fp8 [128, (K//256)*(M//128)*256],
    ordered m-tile-major: blocks (mt, g) of 256 cols = [i(2), m(128)]."""
    K, M = W.shape
    G, MT = K // 256, M // P
    Wf8 = W.astype(NP_FP8)
    # [g, i, p, mt, m] -> [p, mt, g, i, m]
    t = Wf8.reshape(G, 2, P, MT, P).transpose(2, 3, 0, 1, 4)
    return np.ascontiguousarray(t.reshape(P, MT * G * 2 * P))


def _pack_plain(W, kt_major=False):
    """[K, M] fp32 -> bf16 [128, KT*MT*128].

    kt_major=False: col = mt*(KT*P) + kt*P + m  (fc1: ht-major, contraction
    tiles inner).  kt_major=True: col = kt*(MT*P) + mt*P + m  (fc2: streamed
    by contraction tile, all out-tiles inner)."""
    K, M = W.shape
    KT, MT = K // P, M // P
    r = W.astype(NP_BF16).reshape(KT, P, MT, P)
    if kt_major:
        t = r.transpose(1, 0, 2, 3)
    else:
        t = r.transpose(1, 2, 0, 3)
    return np.ascontiguousarray(t.reshape(P, MT * KT * P))


def _prep_inputs(x, ln1_w, ln1_b, w_qkv, w_proj, ln2_w, ln2_b, w_fc1, w_fc2):
    xf = np.asarray(x, np.float32).reshape(TOKS, D)
    ln1_w = np.asarray(ln1_w, np.float32)
    ln1_b = np.asarray(ln1_b, np.float32)
    ln2_w = np.asarray(ln2_w, np.float32)
    ln2_b = np.asarray(ln2_b, np.float32)

    # fold ln1 scale + softmax scale into w_qkv
    wq = np.asarray(w_qkv, np.float32).copy()
    wq[:, 0:D] *= 0.125
    wqs = ln1_w[:, None] * wq
    bq = ln1_b @ wq

    # full x, fp8, c-tile-major feature layout
    xq8_q = xf.astype(NP_FP8)
    xq8 = np.ascontiguousarray(
        xq8_q.reshape(TOKS, CT, P).transpose(2, 1, 0).reshape(P, CT * TOKS))

    wproj_p = _pack_dr(np.asarray(w_proj, np.float32))
    w1s = ln2_w[:, None] * np.asarray(w_fc1, np.float32)
    wfc1_p = _pack_plain(w1s)
    bfc1_v = ln2_b @ np.asarray(w_fc1, np.float32)
    bfc1_p = np.ascontiguousarray(bfc1_v.reshape(32, P).T)
    wfc2_p = _pack_plain(np.asarray(w_fc2, np.float32))

    pp, jj = np.meshgrid(np.arange(P), np.arange(P), indexing="ij")
    cm = np.where(pp <= jj, 0.0, -1e30).astype(NP_BF16)
    idb_m = np.eye(P, dtype=np.float32).astype(NP_BF16)

    common = {
        "xq8": xq8, "wproj": wproj_p,
        "wfc1": wfc1_p, "bfc1": bfc1_p, "wfc2": wfc2_p,
        "cmask": cm, "idb": idb_m,
    }
    in_maps = []
    for i in range(NC):
        m = dict(common)
        m["xT"] = np.ascontiguousarray(xf[TB * i:TB * (i + 1)].T)
        qsl = slice(i * P, (i + 1) * P)
        ksl = slice(D + i * P, D + (i + 1) * P)
        vsl = slice(2 * D + i * P, 2 * D + (i + 1) * P)
        Wqk = np.concatenate([wqs[:, qsl], wqs[:, ksl]], axis=1)  # [D, 256]
        m["wqk"] = _pack_dr(Wqk)
        cs_qk = Wqk.astype(NP_FP8).astype(np.float32).sum(0)      # [256]
        b_qk = np.concatenate([bq[qsl], bq[ksl]])
        m["cbqk"] = np.ascontiguousarray(
            np.stack([cs_qk, b_qk]).astype(NP_BF16))
        Wv8 = wqs[:, vsl].astype(NP_FP8)                          # [D, 128]
        m["wv"] = np.ascontiguousarray(
            Wv8.reshape(4, 2, P, P).transpose(2, 0, 1, 3).reshape(P, 8 * P))
        m["cbv"] = np.ascontiguousarray(
            np.stack([Wv8.astype(np.float32).sum(0),
                      bq[vsl]]).astype(NP_BF16))
        in_maps.append(m)
    return in_maps


def _get_runner():
    if "runner" in _cache:
        return _cache["runner"]
    import jax
    from jax.sharding import Mesh, PartitionSpec, NamedSharding
    from jax.experimental.shard_map import shard_map
    from concourse import bass2jax

    nc = _cache.get("nc")
    if nc is None:
        nc = _cache["nc"] = _build()
    bass2jax.install_neuronx_cc_hook()
    partition_name = nc.partition_id_tensor.name if nc.partition_id_tensor else None
    in_names, out_names, out_avals, zero_outs = [], [], [], []
    for alloc in nc.m.functions[0].allocations:
        if not isinstance(alloc, mybir.MemoryLocationSet):
            continue
        name = alloc.memorylocations[0].name
        if alloc.kind == "ExternalInput":
            if name != partition_name:
                in_names.append(name)
        elif alloc.kind == "ExternalOutput":
            out_names.append(name)
            shape = tuple(alloc.tensor_shape)
            dtype = mybir.dt.np(alloc.dtype)
            out_avals.append(jax.core.ShapedArray(shape, dtype))
            zero_outs.append(np.zeros(shape, dtype))
    n_params = len(in_names)
    all_in_names = in_names + out_names + ([partition_name] if partition_name else [])

    def _body(*args):
        operands = list(args)
        if partition_name is not None:
            operands.append(bass2jax.partition_id_tensor())
        outs = bass2jax._bass_exec_p.bind(
            *operands, out_avals=tuple(out_avals), in_names=tuple(all_in_names),
            out_names=tuple(out_names), lowering_input_output_aliases=(),
            sim_require_finite=True, sim_require_nnan=True, nc=nc)
        return tuple(outs)

    devices = jax.devices()[:NC]
    mesh = Mesh(np.asarray(devices), ("core",))
    nin = n_params + len(out_names)
    sharded = jax.jit(shard_map(
        _body, mesh=mesh, in_specs=(PartitionSpec("core"),) * nin,
        out_specs=(PartitionSpec("core"),) * len(out_names), check_rep=False))
    sh = NamedSharding(mesh, PartitionSpec("core"))
    dev_zeros = [
        jax.device_put(np.zeros((NC * z.shape[0], *z.shape[1:]), z.dtype), sh)
        for z in zero_outs
    ]
    runner = (sharded, in_names, out_names, out_avals, sh, dev_zeros)
    _cache["runner"] = runner
    return runner


def kernel(**inputs):
    import jax
    sharded, in_names, out_names, out_avals, sh, dev_zeros = _get_runner()
    in_maps = _prep_inputs(**inputs)
    concat_in = [np.concatenate([in_maps[c][nm] for c in range(NC)], axis=0)
                 for nm in in_names]
    dev_in = [jax.device_put(a, sh) for a in concat_in]
    out_arrs = sharded(*dev_in, *dev_zeros)
    got = {nm: np.asarray(out_arrs[i]).reshape(NC, *out_avals[i].shape)
           for i, nm in enumerate(out_names)}
    out = np.empty((TOKS, D), np.float32)
    for i in range(NC):
        out[TB * i:TB * (i + 1)] = got["out_t"][i].T
    return out.reshape(2, 2048, D)


if __name__ == "__main__":
    rng = np.random.default_rng(0)
    ins = {
        "x": rng.standard_normal((2, 2048, D), dtype=np.float32),
        "ln1_w": np.ones(D, np.float32),
        "ln1_b": np.zeros(D, np.float32),
        "w_qkv": (rng.standard_normal((D, 3 * D), dtype=np.float32) / 32.0),
        "w_proj": (rng.standard_normal((D, D), dtype=np.float32) / 32.0),
        "ln2_w": np.ones(D, np.float32),
        "ln2_b": np.zeros(D, np.float32),
        "w_fc1": (rng.standard_normal((D, FF), dtype=np.float32) / 32.0),
        "w_fc2": (rng.standard_normal((FF, D), dtype=np.float32) / 64.0),
    }
    out = kernel(**ins)
    print("kernel out", out.shape, out.dtype, float(np.abs(out).mean()))
